# revision 46
# baseline (speedup 1.0000x reference)
"""Embedding lookup kernel for Trainium2 (8 NeuronCores, SPMD).

An embedding lookup IS a row gather: out[b, s, :] = weight[x[b, s], :].
Tokens are sharded 8 ways (1024 contiguous tokens per core); every core
keeps the full table in its DRAM. SHIPPED VARIANT (kernel() ->
build_nc_v6 shared_idx/shared_g/lean + staggered dual-engine writebacks):

Per core:
  1. One HWDGE DMA loads idx [128, 8] int32 into SBUF (idx[p, j] = token
     p*8 + j). Completion latency ~1.4-3us.
  2. A WARMUP indirect op (zeroed idx -> scratch) runs inside the idx
     latency window; it absorbs the first-op cold cost (~0.3-0.5us) so
     all real ops dispatch at the steady ~1.41us cadence.
  3. EIGHT indirect_dma_start ops (InstDMACopy + DynamicAccessPatternInfo
     on the qPoolDynamic SWDGE queue): op j gathers w[idx[p, j]] -> 512B
     row into g[p, j*128:(j+1)*128] for all 128 partitions. The HW
     indirect1d expansion consumes exactly ONE index per DEST PARTITION
     per op (one desc per index, elem = the partition's contiguous span),
     so 1024 rows need 8 ops; each occupies the Pool dispatch ~1.41us
     SERIALLY (SEQ<->Q7 command handshake; the queue attr on InstDMACopy
     does NOT route across SWDGE queues - only custom ucode instructions
     carry queue_num).
  4. Writebacks staggered over BOTH HWDGE engines - (0-3) sync, (4,5)
     act, (6) sync, (7) act - each fired the moment its last column's
     completion sem lands (lean waits: same-ring FIFO per SDMA engine
     means a group's last op's sem implies the whole group's data is in
     SBUF). The final writeback is a single 64KB column, minimizing the
     post-gather tail. All DMAs keep .then_inc (NEFF build requires a sem
     update per DMA).

One-shot cost on HW ~23.3-23.5us: NEFF preamble ~5.9 (engine barriers +
TPB base loads + input-ready event; framework-emitted, not removable
from bass) + idx ~2.6 + 8x1.41 dispatch + last-op drain ~1.7 + final wb
~1.2 + epilogue ~1.1. Run-to-run variance +-1.5us (preamble/idx luck).

Measured dead ends (traces in this session):
- ONE batched indirect op ([P, TPP] idx, flat 2D dest) runs 15.2us total
  but gathers w[idx[p,0]+j] (contiguous run per partition) - wrong for
  arbitrary tokens. 3D dest APs scramble; DRAM->DRAM dest crashes the
  runtime.
- dma_gather (ucode): mlp LOAD_LIB costs 8.8us one-shot even hoisted
  first (v8 ~24.9); loop-amortized v4-line numbers do not transfer.
- v10 hybrid (indirect ops supposedly overlapping the lib load): the
  LOAD_LIB blocks even base-firmware SWDGE desc-gen until loaded (30.5us).
- ap_gather (vocab-sharded SBUF table): ~27ns/token at d=1 (41us);
  >=2MB DMAs concurrent with a library load starve the Q7 loader
  (2.3us -> 43us).

loop_m > 1 builds the timing-harness variant (cross-iteration
serialization) used by bench scripts.
"""

import contextlib

import numpy as np

import concourse.bass as bass
from concourse import bacc, library_config, mybir
from concourse.bass_utils import run_bass_kernel_spmd

N_CORES = 8
B, S = 2, 4096
VOCAB, DIM = 32000, 128
P = 128
TOKENS = B * S                      # 8192
TPC = TOKENS // N_CORES             # 1024 tokens per core
TPP = TPC // P                      # 8 tokens per partition
IDX_COLS = TPC // 16                # 64 int16 idxs per partition row


def build_nc(loop_m: int = 1):
    # Skip the Bass-constructor entry barrier (gates the first DMA behind
    # all engines' init); restore the method right after construction.
    orig_barrier = bass.Bass.all_engine_barrier
    bass.Bass.all_engine_barrier = lambda self, *a, **k: None
    try:
        nc = bacc.Bacc(
            None, target_bir_lowering=False, dynamic_dma_scratch_size=32768
        )
    finally:
        bass.Bass.all_engine_barrier = orig_barrier

    x = nc.dram_tensor("x", [P, IDX_COLS], mybir.dt.int16, kind="ExternalInput")
    w = nc.dram_tensor("weight", [VOCAB, DIM], mybir.dt.float32, kind="ExternalInput")
    out = nc.dram_tensor("out", [P, TPP, DIM], mybir.dt.float32, kind="ExternalOutput")

    with contextlib.ExitStack() as ctx:
        idx_tile = ctx.enter_context(
            nc.sbuf_tensor("idx_tile", [P, IDX_COLS], mybir.dt.int16)
        )
        g = ctx.enter_context(nc.sbuf_tensor("g", [P, TPP, DIM], mybir.dt.float32))
        dummy_idx = ctx.enter_context(
            nc.sbuf_tensor("dummy_idx", [P, 8], mybir.dt.int16)
        )
        scratch = ctx.enter_context(
            nc.sbuf_tensor("scratch", [P, 1, DIM], mybir.dt.float32)
        )
        s_idx = ctx.enter_context(nc.semaphore("s_idx"))
        s_warm = ctx.enter_context(nc.semaphore("s_warm"))
        s_ms = ctx.enter_context(nc.semaphore("s_ms"))
        s_g = ctx.enter_context(nc.semaphore("s_g"))
        s_out = ctx.enter_context(nc.semaphore("s_out"))

        # Hoist the num_idxs register materialization off the critical path
        # (otherwise the mov lands after the s_idx wait).
        n_reg = nc.gpsimd.to_reg(TPC)

        # Warmup gather (128 zero indices), hidden inside the idx-DMA latency
        # window; also pulls the gpsimd library load off the critical path.
        nc.gpsimd.memset(dummy_idx[:], 0).then_inc(s_ms, 1)
        nc.gpsimd.wait_ge(s_ms, 1)
        nc.gpsimd.dma_gather(
            scratch[:], w[:], dummy_idx[:], P, P, DIM
        ).then_inc(s_warm, 16)

        # loop_m > 1 is the timing-harness mode: repeat the body with full
        # cross-iteration serialization (iter k+1's idx load waits for iter
        # k's writeback) so wall-time deltas measure per-iteration latency.
        for k in range(loop_m):
            if k > 0:
                nc.sync.wait_ge(s_out, 16 * k)
            nc.sync.dma_start(idx_tile[:], x[:]).then_inc(s_idx, 16)
            nc.gpsimd.wait_ge(s_idx, 16 * (k + 1))
            nc.gpsimd.dma_gather(
                g[:], w[:], idx_tile[:], TPC, n_reg, DIM
            ).then_inc(s_g, 16)
            nc.sync.wait_ge(s_g, 16 * (k + 1))
            nc.sync.dma_start(out[:], g[:]).then_inc(s_out, 16)
    nc.compile()
    return nc


def build_nc_v3(loop_m: int = 1):
    """v3: writeback via a prepared dma_scatter_add with iota indices.

    The scatter's descriptors (SBUF g -> DRAM out rows 0..1023) are generated
    on the Pool engine while the gather's data is still draining, then fired
    with trigger_dma as soon as the gather's completion semaphore arrives —
    removing the HWDGE dispatch from the tail. out rows are pre-zeroed by the
    runtime, so += is =. Output lands in natural token order [1024, 128].
    """
    orig_barrier = bass.Bass.all_engine_barrier
    bass.Bass.all_engine_barrier = lambda self, *a, **k: None
    try:
        nc = bacc.Bacc(
            None, target_bir_lowering=False, dynamic_dma_scratch_size=32768
        )
    finally:
        bass.Bass.all_engine_barrier = orig_barrier

    x = nc.dram_tensor("x", [P, IDX_COLS], mybir.dt.int16, kind="ExternalInput")
    wbx = nc.dram_tensor("wb_idx", [P, IDX_COLS], mybir.dt.int16, kind="ExternalInput")
    w = nc.dram_tensor("weight", [VOCAB, DIM], mybir.dt.float32, kind="ExternalInput")
    out = nc.dram_tensor("out", [TPC, DIM], mybir.dt.float32, kind="ExternalOutput")

    with contextlib.ExitStack() as ctx:
        idx_tile = ctx.enter_context(
            nc.sbuf_tensor("idx_tile", [P, IDX_COLS], mybir.dt.int16)
        )
        wbx_tile = ctx.enter_context(
            nc.sbuf_tensor("wbx_tile", [P, IDX_COLS], mybir.dt.int16)
        )
        g = ctx.enter_context(nc.sbuf_tensor("g", [P, TPP, DIM], mybir.dt.float32))
        dummy_idx = ctx.enter_context(
            nc.sbuf_tensor("dummy_idx", [P, 8], mybir.dt.int16)
        )
        scratch = ctx.enter_context(
            nc.sbuf_tensor("scratch", [P, 1, DIM], mybir.dt.float32)
        )
        s_idx = ctx.enter_context(nc.semaphore("s_idx"))
        s_wbx = ctx.enter_context(nc.semaphore("s_wbx"))
        s_warm = ctx.enter_context(nc.semaphore("s_warm"))
        s_ms = ctx.enter_context(nc.semaphore("s_ms"))
        s_g = ctx.enter_context(nc.semaphore("s_g"))
        s_wb = ctx.enter_context(nc.semaphore("s_wb"))
        s_prep = ctx.enter_context(nc.semaphore("s_prep"))

        nc.gpsimd.memset(dummy_idx[:], 0).then_inc(s_ms, 1)
        nc.gpsimd.wait_ge(s_ms, 1)
        nc.gpsimd.dma_gather(
            scratch[:], w[:], dummy_idx[:], P, P, DIM
        ).then_inc(s_warm, 16)

        nc.sync.dma_start(idx_tile[:], x[:]).then_inc(s_idx, 16)
        nc.sync.dma_start(wbx_tile[:], wbx[:]).then_inc(s_wbx, 16)

        for k in range(loop_m):
            if k > 0:
                nc.sync.wait_ge(s_wb, 16 * k)
                nc.sync.dma_start(idx_tile[:], x[:]).then_inc(s_idx, 16)
            nc.gpsimd.wait_ge(s_idx, 16 * (k + 1))
            nc.gpsimd.dma_gather(g[:], w[:], idx_tile[:], TPC, TPC, DIM).then_inc(
                s_g, 16
            )
            if k == 0:
                nc.gpsimd.wait_ge(s_wbx, 16)
            nc.gpsimd.dma_scatter_add(
                out[:], g[:], wbx_tile[:], TPC, TPC, DIM,
                prepare_only=True, sem=s_wb,
            ).then_inc(s_prep, 1)
            nc.gpsimd.wait_ge(s_prep, k + 1)
            nc.gpsimd.wait_ge(s_g, 16 * (k + 1))
            nc.gpsimd.trigger_dma(count=1)
        nc.gpsimd.wait_ge(s_wb, 16 * loop_m)
    nc.compile()
    return nc


def build_nc_v4(loop_m: int = 1, sizes=(512, 512), wb_engines=("sync",),
                n_queues: int = 1, warm_queues: int | None = None,
                warm_in_loop: bool = False, single_packet: bool = True,
                wb_groups=None):
    """v4: gather + writeback split into pipelined chunks of `sizes` tokens
    (each a multiple of 128). Chunk c's HWDGE writeback overlaps chunk c+1's
    gather desc-gen/drain, at the price of an extra ~1us SWDGE fixed overhead
    per extra chunk. wb_engines: round-robin engines for the writebacks
    ("sync" = SP, "act" = Activation). n_queues > 1 round-robins the gathers
    over that many SWDGE queues."""
    assert sum(sizes) == TPC and all(s % 128 == 0 for s in sizes)
    orig_barrier = bass.Bass.all_engine_barrier
    bass.Bass.all_engine_barrier = lambda self, *a, **k: None
    try:
        nc = bacc.Bacc(
            None, target_bir_lowering=False, dynamic_dma_scratch_size=32768,
            num_swdge_queues=n_queues, use_seq_codegen=seq_codegen,
        )
    finally:
        bass.Bass.all_engine_barrier = orig_barrier

    x = nc.dram_tensor("x", [P, IDX_COLS], mybir.dt.int16, kind="ExternalInput")
    w = nc.dram_tensor("weight", [VOCAB, DIM], mybir.dt.float32, kind="ExternalInput")
    out = nc.dram_tensor("out", [P, TPP, DIM], mybir.dt.float32, kind="ExternalOutput")

    chunks = len(sizes)
    bounds = [0]
    for s in sizes:
        bounds.append(bounds[-1] + s)

    with contextlib.ExitStack() as ctx:
        idx_tile = ctx.enter_context(
            nc.sbuf_tensor("idx_tile", [P, IDX_COLS], mybir.dt.int16)
        )
        g = ctx.enter_context(nc.sbuf_tensor("g", [P, TPP, DIM], mybir.dt.float32))
        dummy_idx = ctx.enter_context(
            nc.sbuf_tensor("dummy_idx", [P, 8], mybir.dt.int16)
        )
        scratch = ctx.enter_context(
            nc.sbuf_tensor("scratch", [P, max(n_queues, 1), DIM], mybir.dt.float32)
        )
        s_idx = ctx.enter_context(nc.semaphore("s_idx"))
        s_warms = [
            ctx.enter_context(nc.semaphore(f"s_warm{q}"))
            for q in range(max(warm_queues if warm_queues is not None else n_queues, 1))
        ]
        s_ms = ctx.enter_context(nc.semaphore("s_ms"))
        s_gs = [ctx.enter_context(nc.semaphore(f"s_g{c}")) for c in range(chunks)]
        s_out = ctx.enter_context(nc.semaphore("s_out"))

        n_regs = {}
        for s in dict.fromkeys(sizes):
            n_regs[s] = nc.gpsimd.to_reg(s)

        if warm_queues is None:
            # One warmup only: the q0 warmup absorbs the library load + ucode
            # cold cost. Extra per-queue warmups measured ~1.4us each and run
            # serially on the Pool engine, overrunning the ~2.3us idx-DMA
            # window — a guaranteed delay for an unproven per-queue saving.
            warm_queues = 1

        def emit_warmups():
            # One dummy gather per SWDGE queue: warms the ucode path, the
            # per-queue doorbell/ring state, and (queue 0, first) pulls the
            # library load off the critical path. Queue 0 gets the full
            # 128-idx warmup; the rest use 16 idxs (fixed cost dominates).
            for q in range(warm_queues):
                if q == 0:
                    nc.gpsimd.dma_gather(
                        scratch[:, 0:1, :], w[:], dummy_idx[:], P, P, DIM,
                        queue_num=0,
                    ).then_inc(s_warms[0], 16)
                else:
                    nc.gpsimd.dma_gather(
                        scratch[:, q : q + 1, :], w[:], dummy_idx[:, :1], 16, 16,
                        DIM, queue_num=q,
                    ).then_inc(s_warms[q], 16)

        nc.gpsimd.memset(dummy_idx[:], 0).then_inc(s_ms, 1)
        nc.gpsimd.wait_ge(s_ms, 1)
        emit_warmups()

        nc.sync.dma_start(idx_tile[:], x[:]).then_inc(s_idx, 16)

        engines = {"sync": nc.sync, "act": nc.scalar}

        n_wbs = len(wb_groups) if wb_groups else chunks
        for k in range(loop_m):
            if k > 0:
                nc.sync.wait_ge(s_out, 16 * n_wbs * k)
                nc.sync.dma_start(idx_tile[:], x[:]).then_inc(s_idx, 16)
            if warm_in_loop and k > 0:
                emit_warmups()
            nc.gpsimd.wait_ge(s_idx, 16 * (k + 1))
            for c in range(chunks):
                j0, j1 = bounds[c] // P, bounds[c + 1] // P
                nc.gpsimd.dma_gather(
                    g[:, j0:j1, :],
                    w[:],
                    idx_tile[:, bounds[c] // 16 : bounds[c + 1] // 16],
                    sizes[c],
                    n_regs[sizes[c]],
                    DIM,
                    queue_num=c % n_queues,
                    single_packet=single_packet,
                ).then_inc(s_gs[c], 16)
            groups = wb_groups or [(c,) for c in range(chunks)]
            for gi, grp in enumerate(groups):
                j0 = bounds[grp[0]] // P
                j1 = bounds[grp[-1] + 1] // P
                eng = engines[wb_engines[gi % len(wb_engines)]]
                for c in grp:
                    eng.wait_ge(s_gs[c], 16 * (k + 1))
                eng.dma_start(
                    out[:, j0:j1, :], g[:, j0:j1, :]
                ).then_inc(s_out, 16)
    nc.compile()
    return nc


def build_nc_v4b(loop_m: int = 1):
    return build_nc_v4(loop_m, sizes=(640, 384))


def build_nc_v4c(loop_m: int = 1):
    return build_nc_v4(loop_m, sizes=(512, 256, 256))


def build_nc_v4d(loop_m: int = 1):
    return build_nc_v4(loop_m, sizes=(256, 256, 256, 256))


def build_nc_v4c2(loop_m: int = 1):
    return build_nc_v4(loop_m, sizes=(512, 256, 256), wb_engines=("sync", "act"))


def build_nc_v4asc(loop_m: int = 1):
    return build_nc_v4(loop_m, sizes=(256, 256, 512))


def build_nc_v4ascq(loop_m: int = 1):
    return build_nc_v4(loop_m, sizes=(256, 256, 512), n_queues=3)


def build_nc_v4ascq_w(loop_m: int = 1):
    # probe: per-queue warmups re-run inside every loop iteration
    return build_nc_v4(loop_m, sizes=(256, 256, 512), n_queues=3,
                       warm_in_loop=True)


def build_nc_v4ascq2(loop_m: int = 1):
    return build_nc_v4(loop_m, sizes=(256, 256, 512), n_queues=2)


def build_nc_v4eq(loop_m: int = 1):
    return build_nc_v4(loop_m, sizes=(128, 128, 256, 512), n_queues=4)


def build_nc_v4gq(loop_m: int = 1):
    return build_nc_v4(loop_m, sizes=(128, 128, 128, 128, 512), n_queues=4)


def _probe_builder(loop_m: int, *, no_idx: bool = False, no_wb: bool = False,
                   sizes=(128, 128, 256, 512), n_queues: int = 4):
    """Timing probes: v4eq with the per-iteration idx DMA and/or the
    writebacks removed, to decompose per-iteration time on HW."""
    orig_barrier = bass.Bass.all_engine_barrier
    bass.Bass.all_engine_barrier = lambda self, *a, **k: None
    try:
        nc = bacc.Bacc(
            None, target_bir_lowering=False, dynamic_dma_scratch_size=32768,
            num_swdge_queues=n_queues, use_seq_codegen=seq_codegen,
        )
    finally:
        bass.Bass.all_engine_barrier = orig_barrier

    x = nc.dram_tensor("x", [P, IDX_COLS], mybir.dt.int16, kind="ExternalInput")
    w = nc.dram_tensor("weight", [VOCAB, DIM], mybir.dt.float32, kind="ExternalInput")
    out = nc.dram_tensor("out", [P, TPP, DIM], mybir.dt.float32, kind="ExternalOutput")

    chunks = len(sizes)
    bounds = [0]
    for s in sizes:
        bounds.append(bounds[-1] + s)

    with contextlib.ExitStack() as ctx:
        idx_tile = ctx.enter_context(
            nc.sbuf_tensor("idx_tile", [P, IDX_COLS], mybir.dt.int16)
        )
        g = ctx.enter_context(nc.sbuf_tensor("g", [P, TPP, DIM], mybir.dt.float32))
        dummy_idx = ctx.enter_context(
            nc.sbuf_tensor("dummy_idx", [P, 8], mybir.dt.int16)
        )
        scratch = ctx.enter_context(
            nc.sbuf_tensor("scratch", [P, 1, DIM], mybir.dt.float32)
        )
        s_idx = ctx.enter_context(nc.semaphore("s_idx"))
        s_warm = ctx.enter_context(nc.semaphore("s_warm"))
        s_ms = ctx.enter_context(nc.semaphore("s_ms"))
        s_gs = [ctx.enter_context(nc.semaphore(f"s_g{c}")) for c in range(chunks)]
        s_out = ctx.enter_context(nc.semaphore("s_out"))

        n_regs = {}
        for s in dict.fromkeys(sizes):
            n_regs[s] = nc.gpsimd.to_reg(s)

        nc.gpsimd.memset(dummy_idx[:], 0).then_inc(s_ms, 1)
        nc.gpsimd.wait_ge(s_ms, 1)
        nc.gpsimd.dma_gather(
            scratch[:], w[:], dummy_idx[:], P, P, DIM, queue_num=0
        ).then_inc(s_warm, 16)

        nc.sync.dma_start(idx_tile[:], x[:]).then_inc(s_idx, 16)

        for k in range(loop_m):
            if no_idx:
                if k > 0:
                    # serialize iterations + WAR-protect g without an idx DMA
                    nc.gpsimd.wait_ge(
                        s_out if not no_wb else s_gs[-1],
                        (16 * chunks * k) if not no_wb else 16 * k,
                    )
                nc.gpsimd.wait_ge(s_idx, 16)
            else:
                if k > 0:
                    if no_wb:
                        for c in range(chunks):
                            nc.sync.wait_ge(s_gs[c], 16 * k)
                    else:
                        nc.sync.wait_ge(s_out, 16 * chunks * k)
                    nc.sync.dma_start(idx_tile[:], x[:]).then_inc(s_idx, 16)
                nc.gpsimd.wait_ge(s_idx, 16 * (k + 1))
            for c in range(chunks):
                j0, j1 = bounds[c] // P, bounds[c + 1] // P
                nc.gpsimd.dma_gather(
                    g[:, j0:j1, :],
                    w[:],
                    idx_tile[:, bounds[c] // 16 : bounds[c + 1] // 16],
                    sizes[c],
                    n_regs[sizes[c]],
                    DIM,
                    queue_num=c % n_queues,
                ).then_inc(s_gs[c], 16)
            if not no_wb:
                for c in range(chunks):
                    j0, j1 = bounds[c] // P, bounds[c + 1] // P
                    nc.sync.wait_ge(s_gs[c], 16 * (k + 1))
                    nc.sync.dma_start(
                        out[:, j0:j1, :], g[:, j0:j1, :]
                    ).then_inc(s_out, 16)
        if no_wb:
            # write the output once at the end so the ExternalOutput has a
            # writer (walrus requires semaphore updates per DMA anyway)
            for c in range(chunks):
                nc.sync.wait_ge(s_gs[c], 16 * loop_m)
            nc.sync.dma_start(out[:], g[:]).then_inc(s_out, 16)
    nc.compile()
    return nc


def build_nc_v4mq(loop_m: int = 1):
    # merged writeback for the two 128-token primer chunks
    return build_nc_v4(loop_m, sizes=(128, 128, 256, 512), n_queues=4,
                       wb_groups=[(0, 1), (2,), (3,)])


def build_nc_v4sp(loop_m: int = 1):
    return build_nc_v4(loop_m, sizes=(128, 128, 256, 512), n_queues=4,
                       single_packet=False)


def build_nc_v4hq(loop_m: int = 1):
    # tail-light ordering: big chunk third, tiny chunk last so the final
    # drain + writeback are short
    return build_nc_v4(loop_m, sizes=(128, 256, 512, 128), n_queues=4)


def build_nc_v4eq_noidx(loop_m: int = 1):
    return _probe_builder(loop_m, no_idx=True)


def build_nc_v4eq_nowb(loop_m: int = 1):
    return _probe_builder(loop_m, no_wb=True)


def build_nc_v4eq_gonly(loop_m: int = 1):
    return _probe_builder(loop_m, no_idx=True, no_wb=True)


def build_nc_v4e(loop_m: int = 1):
    return build_nc_v4(loop_m, sizes=(128, 128, 256, 512))


def build_nc_v4cq(loop_m: int = 1):
    return build_nc_v4(loop_m, sizes=(512, 256, 256), n_queues=3)


def build_nc_v4q2(loop_m: int = 1):
    return build_nc_v4(loop_m, sizes=(512, 512), n_queues=2)


def build_nc_v4dq(loop_m: int = 1):
    return build_nc_v4(loop_m, sizes=(256, 256, 256, 256), n_queues=4)


def build_nc_v5(loop_m: int = 1, sizes=(TPP,), wb_groups=None):
    """v5: ONE batched vector-indirect DMA (InstDMACopy on qPoolDynamic)
    instead of gpsimd dma_gather ucode.

    dma_gather needs the `mlp` gpsimd library; its LOAD_LIB DMA (~8.8us) +
    warmup gather (~1.6us) sit on the one-shot critical path (the graded
    metric is a single NEFF execution, so the loop-harness amortization the
    v4 line was tuned for never happens). indirect_dma_start lowers to a
    plain DMA with DynamicAccessPatternInfo - descriptor expansion happens
    in the SWDGE base path, no library load, no ucode warmup.

    Index layout: idx_tile[p, j] = token p*TPP + j (int32, [128, 8]); each
    index gathers one contiguous 512B row of w; row i (offset-AP C-order
    p*TPP+j) lands at output block i = g[p, j, :]. Host reshape(TPC, DIM)
    restores token order. `sizes` splits the gather into chunks of TPP
    columns each so chunk c's HWDGE writeback overlaps chunk c+1's drain.
    """
    assert sum(sizes) == TPP
    orig_barrier = bass.Bass.all_engine_barrier
    bass.Bass.all_engine_barrier = lambda self, *a, **k: None
    try:
        nc = bacc.Bacc(
            None, target_bir_lowering=False, dynamic_dma_scratch_size=32768
        )
    finally:
        bass.Bass.all_engine_barrier = orig_barrier

    x = nc.dram_tensor("x", [P, TPP], mybir.dt.int32, kind="ExternalInput")
    w = nc.dram_tensor("weight", [VOCAB, DIM], mybir.dt.float32, kind="ExternalInput")
    out = nc.dram_tensor("out", [P, TPP, DIM], mybir.dt.float32, kind="ExternalOutput")

    chunks = len(sizes)
    bounds = [0]
    for s in sizes:
        bounds.append(bounds[-1] + s)

    with contextlib.ExitStack() as ctx:
        idx_tile = ctx.enter_context(
            nc.sbuf_tensor("idx_tile", [P, TPP], mybir.dt.int32)
        )
        g = ctx.enter_context(nc.sbuf_tensor("g", [P, TPP, DIM], mybir.dt.float32))
        s_idx = ctx.enter_context(nc.semaphore("s_idx"))
        s_gs = [ctx.enter_context(nc.semaphore(f"s_g{c}")) for c in range(chunks)]
        s_out = ctx.enter_context(nc.semaphore("s_out"))

        groups = wb_groups or [(c,) for c in range(chunks)]
        n_wbs = len(groups)
        for k in range(loop_m):
            if k > 0:
                nc.sync.wait_ge(s_out, 16 * n_wbs * k)
            nc.sync.dma_start(idx_tile[:], x[:]).then_inc(s_idx, 16)
            nc.gpsimd.wait_ge(s_idx, 16 * (k + 1))
            for c in range(chunks):
                j0, j1 = bounds[c], bounds[c + 1]
                nc.gpsimd.indirect_dma_start(
                    out=g[:, j0:j1, :],
                    out_offset=None,
                    in_=w[:],
                    in_offset=bass.IndirectOffsetOnAxis(
                        ap=idx_tile[:, j0:j1], axis=0
                    ),
                ).then_inc(s_gs[c], 16)
            for grp in groups:
                j0, j1 = bounds[grp[0]], bounds[grp[-1] + 1]
                for c in grp:
                    nc.sync.wait_ge(s_gs[c], 16 * (k + 1))
                nc.sync.dma_start(
                    out[:, j0:j1, :], g[:, j0:j1, :]
                ).then_inc(s_out, 16)
    nc.compile()
    return nc


IDXW = 64  # idx tile words/partition; 256B pitch is required for the
           # batched vector-indirect op to fetch one index per dest block
           # (32B pitch degrades to one index per partition + contiguous run)


def build_nc_v7(loop_m: int = 1, wb_groups=((0, TPP),)):
    """v7: ONE batched vector-indirect DMA for all 1024 rows.

    Probed on HW (dbg_walk.py): with the idx tile at 256B partition pitch
    ([P, 64] int32, first TPP words used) and a flat 2D dest [P, TPP*DIM],
    the SWDGE indirect1d expansion consumes exactly one index per 512B dest
    block in (p, j) C-order: g[p, j*DIM:(j+1)*DIM] = w[idx[p, j]]. One Pool
    dispatch (~1.1us) replaces 8 (~11.3us serial).

    wb_groups: list of (j0, j1) column ranges, one HWDGE writeback each,
    gather split per group so wb overlaps the later groups' drains.
    """
    orig_barrier = bass.Bass.all_engine_barrier
    bass.Bass.all_engine_barrier = lambda self, *a, **k: None
    try:
        nc = bacc.Bacc(
            None, target_bir_lowering=False, dynamic_dma_scratch_size=32768
        )
    finally:
        bass.Bass.all_engine_barrier = orig_barrier

    x = nc.dram_tensor("x", [P, IDXW], mybir.dt.int32, kind="ExternalInput")
    w = nc.dram_tensor("weight", [VOCAB, DIM], mybir.dt.float32, kind="ExternalInput")
    out = nc.dram_tensor("out", [P, TPP, DIM], mybir.dt.float32, kind="ExternalOutput")

    with contextlib.ExitStack() as ctx:
        idx_t = ctx.enter_context(nc.sbuf_tensor("idx", [P, IDXW], mybir.dt.int32))
        g_t = ctx.enter_context(
            nc.sbuf_tensor("g", [P, TPP * DIM], mybir.dt.float32)
        )
        s_idx = ctx.enter_context(nc.semaphore("s_idx"))
        s_gs = [ctx.enter_context(nc.semaphore(f"s_g{i}")) for i in range(len(wb_groups))]
        s_out = ctx.enter_context(nc.semaphore("s_out"))

        for k in range(loop_m):
            if k > 0:
                nc.sync.wait_ge(s_out, 16 * len(wb_groups) * k)
            nc.sync.dma_start(idx_t[:], x[:]).then_inc(s_idx, 16)
            nc.gpsimd.wait_ge(s_idx, 16 * (k + 1))
            for gi, (j0, j1) in enumerate(wb_groups):
                nc.gpsimd.indirect_dma_start(
                    out=g_t[:, j0 * DIM : j1 * DIM],
                    out_offset=None,
                    in_=w[:],
                    in_offset=bass.IndirectOffsetOnAxis(
                        ap=idx_t[:, j0:j1], axis=0
                    ),
                ).then_inc(s_gs[gi], 16)
            for gi, (j0, j1) in enumerate(wb_groups):
                nc.sync.wait_ge(s_gs[gi], 16 * (k + 1))
                nc.sync.dma_start(
                    out[:, j0:j1, :], g_t[:, j0 * DIM : j1 * DIM]
                ).then_inc(s_out, 16)
    nc.compile()
    return nc


def build_nc_v6(loop_m: int = 1, n_queues: int = 1,
                wb_groups=((0, 1, 2, 3), (4, 5, 6, 7)),
                col_order=None, shared_idx: bool = False,
                shared_g: bool = False, batched: bool = False,
                wb_engines=("sync",), lean: bool = False,
                ring_alt: bool = False):
    """v6: 8 per-column vector-indirect DMAs (the HW-proven expansion shape:
    ONE index per partition per op - idx[:, j:j+1] -> g[:, j, :]).

    No gpsimd ucode library, no LOAD_LIB (~8.8us) and no warmup (~1.6us) on
    the one-shot critical path. `n_queues` > 1 round-robins the ops across
    qPoolDynamic{i} SWDGE queues (desc-gen parallelizes across queues; the
    InstDMACopy queue attr is patched post-construction). wb_groups: column
    groups per HWDGE writeback, fired as soon as member gathers complete.
    """
    orig_barrier = bass.Bass.all_engine_barrier
    bass.Bass.all_engine_barrier = lambda self, *a, **k: None
    try:
        nc = bacc.Bacc(
            None, target_bir_lowering=False, dynamic_dma_scratch_size=32768,
            num_swdge_queues=n_queues,
        )
    finally:
        bass.Bass.all_engine_barrier = orig_barrier

    # shared_idx: x[p, j] = token p*TPP + j (one contiguous [P, TPP] load).
    # else: x[j, p] = token p*TPP + j so each column-op's [P, 1] idx load is
    # a contiguous 512B DRAM read.
    x_shape = [P, TPP] if shared_idx else [TPP, P]
    x = nc.dram_tensor("x", x_shape, mybir.dt.int32, kind="ExternalInput")
    w = nc.dram_tensor("weight", [VOCAB, DIM], mybir.dt.float32, kind="ExternalInput")
    out = nc.dram_tensor("out", [P, TPP, DIM], mybir.dt.float32, kind="ExternalOutput")

    cols = col_order or list(range(TPP))

    with contextlib.ExitStack() as ctx:
        # Baseline HW-proven shape (tile_scatter_add): per-op dest tile
        # [P, DIM] 2D at offset 0, per-op idx tile [P, 1] 2D at offset 0.
        # shared_idx / shared_g probe whether sliced (offset != 0) 2D APs
        # also work, which collapses the 8 idx loads / 8 writebacks.
        if shared_idx:
            idx_t = ctx.enter_context(
                nc.sbuf_tensor("idx", [P, TPP], mybir.dt.int32)
            )
            idx_aps = [idx_t[:, j : j + 1] for j in range(TPP)]
        else:
            idx_tiles = [
                ctx.enter_context(
                    nc.sbuf_tensor(f"idx{j}", [P, 1], mybir.dt.int32)
                )
                for j in range(TPP)
            ]
            idx_aps = [t[:] for t in idx_tiles]
        if shared_g:
            g_t = ctx.enter_context(
                nc.sbuf_tensor("g", [P, TPP * DIM], mybir.dt.float32)
            )
            g_aps = [g_t[:, j * DIM : (j + 1) * DIM] for j in range(TPP)]
        else:
            g_tiles = [
                ctx.enter_context(
                    nc.sbuf_tensor(f"g{j}", [P, DIM], mybir.dt.float32)
                )
                for j in range(TPP)
            ]
            g_aps = [t[:] for t in g_tiles]
        s_idx = ctx.enter_context(nc.semaphore("s_idx"))
        s_gs = [ctx.enter_context(nc.semaphore(f"s_g{j}")) for j in range(TPP)]
        s_out = ctx.enter_context(nc.semaphore("s_out"))
        if lean:
            warm_idx = ctx.enter_context(
                nc.sbuf_tensor("warm_idx", [P, 1], mybir.dt.int32)
            )
            warm_g = ctx.enter_context(
                nc.sbuf_tensor("warm_g", [P, DIM], mybir.dt.float32)
            )
            s_wm = ctx.enter_context(nc.semaphore("s_wm"))

        n_wbs = len(wb_groups)
        for k in range(loop_m):
            if k > 0:
                nc.sync.wait_ge(s_out, 16 * n_wbs * k)
            if shared_idx:
                nc.sync.dma_start(idx_t[:], x[:]).then_inc(s_idx, 16)
                idx_target = 16
            else:
                for j in range(TPP):
                    nc.sync.dma_start(
                        idx_aps[j], x[j : j + 1, :].rearrange("a b -> b a")
                    ).then_inc(s_idx, 16)
                idx_target = 16 * TPP
            if lean and k == 0:
                # warmup indirect op in the idle idx-latency window: zeroed
                # idx tile -> gathers w[0] into scratch; absorbs the first-op
                # cold cost off the critical path (dispatched before the idx
                # wait, ~6.0-7.4us while idx is still in flight)
                nc.gpsimd.memset(warm_idx[:], 0).then_inc(s_wm, 1)
                nc.gpsimd.wait_ge(s_wm, 1)
                nc.gpsimd.indirect_dma_start(
                    out=warm_g[:],
                    out_offset=None,
                    in_=w[:],
                    in_offset=bass.IndirectOffsetOnAxis(
                        ap=warm_idx[:], axis=0
                    ),
                ).then_inc(s_wm, 16)
            nc.gpsimd.wait_ge(s_idx, idx_target * (k + 1))
            if batched:
                # ONE 2D-everything op: dest [P, TPP*DIM] iterates TPP
                # 512B blocks per partition in lockstep with idx [P, TPP]
                assert shared_idx and shared_g
                for gi, grp in enumerate(wb_groups):
                    j0, j1 = grp[0], grp[-1] + 1
                    inst = nc.gpsimd.indirect_dma_start(
                        out=g_t[:, j0 * DIM : j1 * DIM],
                        out_offset=None,
                        in_=w[:],
                        in_offset=bass.IndirectOffsetOnAxis(
                            ap=idx_t[:, j0:j1], axis=0
                        ),
                    )
                    q = gi % n_queues
                    if q:
                        inst.ins.queue = f"qPoolDynamic{q}"
                    inst.then_inc(s_gs[grp[0]], 16)
            else:
                # every DMA needs a sem update (NEFF build requirement);
                # lean-ness lives on the wait side only
                sem_cols = set(cols)
                cur_ring = [0]

                def swap_ring(q):
                    if q == cur_ring[0]:
                        return
                    cur_ring[0] = q
                    qname = f"qPoolDynamic{q or ''}"
                    nc.gpsimd.isa(
                        nc.isa.Opcode.NEURON_ISA_TPB_OPCODE_PSEUDO_DMASWAP_QUEUE_SET,
                        {
                            "queue_instance_name": list(
                                qname.encode().ljust(32, b"\x00")
                            ),
                            "no_rearm": 0,
                            "reset_semaphores": 0,
                        },
                        struct_name="NEURON_ISA_TPB_PSEUDO_DMA_SWAP_QUEUE_SET_STRUCT",
                        verify=False,
                    )

                for i, j in enumerate(cols):
                    if ring_alt:
                        swap_ring(i % n_queues)
                    inst = nc.gpsimd.indirect_dma_start(
                        out=g_aps[j],
                        out_offset=None,
                        in_=w[:],
                        in_offset=bass.IndirectOffsetOnAxis(
                            ap=idx_aps[j], axis=0
                        ),
                    )
                    q = i % n_queues
                    if q:
                        inst.ins.queue = f"qPoolDynamic{q}"
                    if j in sem_cols:
                        inst.then_inc(s_gs[j], 16)
                if ring_alt:
                    swap_ring(0)
            engs = {"sync": nc.sync, "act": nc.scalar}
            for gi, grp in enumerate(wb_groups):
                eng = engs[wb_engines[gi % len(wb_engines)]]
                # lean: ops share one SWDGE ring; per-SDMA-engine FIFO means
                # the group's LAST op completing implies all earlier ops'
                # data landed, so waiting its sem alone is sound
                wait_cols = (
                    grp[:1]
                    if batched
                    else (grp[-1:] if (lean and not ring_alt) else grp)
                )
                for j in wait_cols:
                    eng.wait_ge(s_gs[j], 16 * (k + 1))
                if shared_g:
                    j0, j1 = grp[0], grp[-1] + 1
                    eng.dma_start(
                        out[:, j0:j1, :], g_t[:, j0 * DIM : j1 * DIM]
                    ).then_inc(s_out, 16)
                else:
                    for j in grp:
                        eng.dma_start(
                            out[:, j : j + 1, :], g_aps[j]
                        ).then_inc(s_out, 16)
    nc.compile()
    return nc


def build_nc_v8(loop_m: int = 1, sizes=(128, 128, 256, 512), n_queues=4,
                wb_groups=((0, 1), (2,), (3,)), warmup=False,
                seq_codegen=False):
    """v8: one-shot-optimized dma_gather. The mlp ucode library load
    (~8.8us) is dispatched as the FIRST Pool instruction (explicit
    load_library) so it fully overlaps the idx DMA + its latency; no
    warmup gather (its ~1.6us serial cost is not worth it one-shot - the
    first real gather pays the cold cost while later chunks' desc-gen
    queues behind it anyway). Chunked gathers on separate SWDGE queues
    (v4 lesson: dispatch is ~70ns when a Q7 core is free; chunks must be
    pow2 sizes; ascending order primes writeback earliest)."""
    assert sum(sizes) == TPC and all(s % 128 == 0 for s in sizes)
    orig_barrier = bass.Bass.all_engine_barrier
    bass.Bass.all_engine_barrier = lambda self, *a, **k: None
    try:
        nc = bacc.Bacc(
            None, target_bir_lowering=False, dynamic_dma_scratch_size=32768,
            num_swdge_queues=n_queues, use_seq_codegen=seq_codegen,
        )
    finally:
        bass.Bass.all_engine_barrier = orig_barrier

    x = nc.dram_tensor("x", [P, IDX_COLS], mybir.dt.int16, kind="ExternalInput")
    w = nc.dram_tensor("weight", [VOCAB, DIM], mybir.dt.float32, kind="ExternalInput")
    out = nc.dram_tensor("out", [P, TPP, DIM], mybir.dt.float32, kind="ExternalOutput")

    chunks = len(sizes)
    bounds = [0]
    for s in sizes:
        bounds.append(bounds[-1] + s)

    with contextlib.ExitStack() as ctx:
        idx_tile = ctx.enter_context(
            nc.sbuf_tensor("idx_tile", [P, IDX_COLS], mybir.dt.int16)
        )
        g = ctx.enter_context(nc.sbuf_tensor("g", [P, TPP, DIM], mybir.dt.float32))
        dummy_idx = ctx.enter_context(
            nc.sbuf_tensor("dummy_idx", [P, 8], mybir.dt.int16)
        )
        scratch = ctx.enter_context(
            nc.sbuf_tensor("scratch", [P, 1, DIM], mybir.dt.float32)
        )
        s_idx = ctx.enter_context(nc.semaphore("s_idx"))
        s_ms = ctx.enter_context(nc.semaphore("s_ms"))
        s_warm = ctx.enter_context(nc.semaphore("s_warm"))
        s_gs = [ctx.enter_context(nc.semaphore(f"s_g{c}")) for c in range(chunks)]
        s_out = ctx.enter_context(nc.semaphore("s_out"))

        # start the ucode library DMA immediately; it runs while the idx
        # DMA + HBM latency elapse
        nc.gpsimd.load_library(library_config.mlp)

        n_regs = {}
        for s in dict.fromkeys(sizes):
            n_regs[s] = nc.gpsimd.to_reg(s)

        if warmup:
            nc.gpsimd.memset(dummy_idx[:], 0).then_inc(s_ms, 1)
            nc.gpsimd.wait_ge(s_ms, 1)
            nc.gpsimd.dma_gather(
                scratch[:], w[:], dummy_idx[:], P, P, DIM, queue_num=0
            ).then_inc(s_warm, 16)

        nc.sync.dma_start(idx_tile[:], x[:]).then_inc(s_idx, 16)

        n_wbs = len(wb_groups)
        for k in range(loop_m):
            if k > 0:
                nc.sync.wait_ge(s_out, 16 * n_wbs * k)
                nc.sync.dma_start(idx_tile[:], x[:]).then_inc(s_idx, 16)
            nc.gpsimd.wait_ge(s_idx, 16 * (k + 1))
            for c in range(chunks):
                j0, j1 = bounds[c] // P, bounds[c + 1] // P
                nc.gpsimd.dma_gather(
                    g[:, j0:j1, :],
                    w[:],
                    idx_tile[:, bounds[c] // 16 : bounds[c + 1] // 16],
                    sizes[c],
                    n_regs[sizes[c]],
                    DIM,
                    queue_num=c % n_queues,
                ).then_inc(s_gs[c], 16)
            for grp in wb_groups:
                j0 = bounds[grp[0]] // P
                j1 = bounds[grp[-1] + 1] // P
                for c in grp:
                    nc.sync.wait_ge(s_gs[c], 16 * (k + 1))
                nc.sync.dma_start(
                    out[:, j0:j1, :], g[:, j0:j1, :]
                ).then_inc(s_out, 16)
    nc.compile()
    return nc


VSH = VOCAB // N_CORES   # 4000 vocab rows per core (vocab-sharded table)
NTOK = 1536              # padded per-core token capacity (exp ~1024, 8.5+sigma)


def build_nc_v9(loop_m: int = 1, n_chunks: int = 1):
    """v9: vocab-sharded ap_gather (SBUF-resident transposed table shard).

    Core c owns vocab rows [c*VSH, (c+1)*VSH); host routes each token to its
    owning core (the sharding_hint's vocab-parallel scheme) and un-permutes
    on the way out. Pipeline per core:
      1. load_library(ap_gather) - small lib, ~2.3us clean
      2. tiny warmup ap_gather; its completion sem doubles as the only
         observable "library loaded" signal, gating the big shard DMA (a
         2MB DMA concurrent with the Q7 library load starves the loader:
         measured 43us vs 2.3us)
      3. WT shard [128 dim, VSH] fp32 DMA -> SBUF (~6.5us at 313GB/s)
      4. one ap_gather: g[dim_p, k] = WT[dim_p, loc_idx[k]], all NTOK tokens
      5. HWDGE writeback g [128, NTOK] -> out DRAM (host transposes back)
    n_chunks > 1 splits 3-5 into vocab-range chunks (host buckets tokens
    per chunk) so shard-DMA / gather / writeback pipeline.
    """
    orig_barrier = bass.Bass.all_engine_barrier
    bass.Bass.all_engine_barrier = lambda self, *a, **k: None
    try:
        nc = bacc.Bacc(
            None, target_bir_lowering=False, dynamic_dma_scratch_size=32768
        )
    finally:
        bass.Bass.all_engine_barrier = orig_barrier

    assert VSH % n_chunks == 0 and NTOK % n_chunks == 0
    vch = VSH // n_chunks
    tch = NTOK // n_chunks

    wt = nc.dram_tensor("wt", [P, VSH], mybir.dt.float32, kind="ExternalInput")
    xw = nc.dram_tensor("xw", [P, NTOK // 16], mybir.dt.int16, kind="ExternalInput")
    out = nc.dram_tensor("out", [P, NTOK], mybir.dt.float32, kind="ExternalOutput")

    with contextlib.ExitStack() as ctx:
        wt_t = ctx.enter_context(nc.sbuf_tensor("wt_t", [P, VSH, 1], mybir.dt.float32))
        idx_t = ctx.enter_context(
            nc.sbuf_tensor("idx_t", [P, NTOK // 16], mybir.dt.int16)
        )
        g_t = ctx.enter_context(nc.sbuf_tensor("g_t", [P, NTOK, 1], mybir.dt.float32))
        dummy = ctx.enter_context(nc.sbuf_tensor("dmy_idx", [P, 1], mybir.dt.int16))
        scr = ctx.enter_context(nc.sbuf_tensor("scr", [P, 16, 1], mybir.dt.float32))
        s_ms = ctx.enter_context(nc.semaphore("s_ms"))
        s_lib = ctx.enter_context(nc.semaphore("s_lib"))
        s_idx = ctx.enter_context(nc.semaphore("s_idx"))
        s_wts = [ctx.enter_context(nc.semaphore(f"s_wt{c}")) for c in range(n_chunks)]
        s_gs = [ctx.enter_context(nc.semaphore(f"s_g{c}")) for c in range(n_chunks)]
        s_out = ctx.enter_context(nc.semaphore("s_out"))

        nc.gpsimd.load_library(library_config.ap_gather)
        # lib-loaded gate: first custom op blocks until the library lands
        nc.gpsimd.memset(dummy[:], 0).then_inc(s_ms, 1)
        nc.gpsimd.wait_ge(s_ms, 1)
        nc.gpsimd.ap_gather(
            scr[:], wt_t[:, :16, :], dummy[:], channels=P, num_elems=16, d=1,
            num_idxs=16,
        ).then_inc(s_lib, 1)

        # idx DMA is tiny - safe concurrent with the library load
        nc.sync.dma_start(idx_t[:], xw[:]).then_inc(s_idx, 16)
        nc.sync.wait_ge(s_lib, 1)

        for k in range(loop_m):
            if k > 0:
                nc.sync.wait_ge(s_out, 16 * n_chunks * k)
            for c in range(n_chunks):
                nc.sync.dma_start(
                    wt_t[:, c * vch : (c + 1) * vch, 0],
                    wt[:, c * vch : (c + 1) * vch],
                ).then_inc(s_wts[c], 16)
            nc.gpsimd.wait_ge(s_idx, 16)
            for c in range(n_chunks):
                nc.gpsimd.wait_ge(s_wts[c], 16 * (k + 1))
                nc.gpsimd.ap_gather(
                    g_t[:, c * tch : (c + 1) * tch, :],
                    wt_t[:, c * vch : (c + 1) * vch, :],
                    idx_t[:, c * (tch // 16) : (c + 1) * (tch // 16)],
                    channels=P,
                    num_elems=vch,
                    d=1,
                    num_idxs=tch,
                ).then_inc(s_gs[c], 1)
            for c in range(n_chunks):
                nc.sync.wait_ge(s_gs[c], k + 1)
                nc.sync.dma_start(
                    out[:, c * tch : (c + 1) * tch],
                    g_t[:, c * tch : (c + 1) * tch, 0],
                ).then_inc(s_out, 16)
    nc.compile()
    return nc


def _v9_in_maps(x_flat: np.ndarray, w: np.ndarray):
    """Route tokens to vocab-owning cores; return in_maps + unpermute info."""
    owner = x_flat // VSH                       # owning core per token
    order = np.argsort(owner, kind="stable")    # token positions grouped by core
    counts = np.bincount(owner, minlength=N_CORES)
    assert counts.max() <= NTOK, f"token bucket overflow: {counts.max()} > {NTOK}"
    in_maps = []
    for c in range(N_CORES):
        sel = order[counts[:c].sum() : counts[: c + 1].sum()]
        loc = (x_flat[sel] - c * VSH).astype(np.int16)
        locp = np.zeros(NTOK, dtype=np.int16)
        locp[: len(sel)] = loc
        t16 = locp.reshape(NTOK // 16, 16).T    # wrap for gpsimd stripes
        in_maps.append({
            "wt": np.ascontiguousarray(w[c * VSH : (c + 1) * VSH, :].T),
            "xw": np.ascontiguousarray(np.tile(t16, (P // 16, 1))),
        })
    return in_maps, order, counts


_NC_CACHE = None


def _wrap_idxs_n(tokens: np.ndarray) -> np.ndarray:
    """dma_gather idx wrap for arbitrary n%16==0 token lists."""
    n = len(tokens)
    t16 = tokens.reshape(n // 16, 16).T.astype(np.int16)
    return np.ascontiguousarray(np.tile(t16, (P // 16, 1)))


def build_nc_v10(loop_m: int = 1, k: int = 4):
    """v10 hybrid: overlap the two gather mechanisms' serial setups.

    Columns 0..k-1 (k*128 tokens) go through library-free vector-indirect
    ops whose ~1.41us/op serial dispatches start as soon as the idx DMA
    lands (~8.6us) - WHILE the mlp ucode library (~8.8us) loads in the
    background. Columns k..7 go through ONE dma_gather dispatched right
    after the indirect ops, by when the library is resident (~15.1us).
    Risk hedged: if indirect desc-gen stalls behind the Q7 library loader,
    the ops simply run at ~15.1 and total time degrades to ~the old 24.

    g2 is 2D [P, TPP*DIM] (3D dest APs break the indirect expansion);
    dma_gather's dest view is rearranged to [P, m, DIM] on the same bytes.
    Token layout matches the shipped v6 kernel: out[p, j] = token p*8+j.
    """
    assert 1 <= k <= 7
    ng = (TPP - k) * P                    # tokens in the dma_gather half
    orig_barrier = bass.Bass.all_engine_barrier
    bass.Bass.all_engine_barrier = lambda self, *a, **k2: None
    try:
        nc = bacc.Bacc(
            None, target_bir_lowering=False, dynamic_dma_scratch_size=32768
        )
    finally:
        bass.Bass.all_engine_barrier = orig_barrier

    x32 = nc.dram_tensor("x32", [P, k], mybir.dt.int32, kind="ExternalInput")
    xw = nc.dram_tensor("xw", [P, ng // 16], mybir.dt.int16, kind="ExternalInput")
    w = nc.dram_tensor("weight", [VOCAB, DIM], mybir.dt.float32, kind="ExternalInput")
    out = nc.dram_tensor("out", [P, TPP, DIM], mybir.dt.float32, kind="ExternalOutput")

    with contextlib.ExitStack() as ctx:
        idx32_t = ctx.enter_context(nc.sbuf_tensor("idx32", [P, k], mybir.dt.int32))
        idxw_t = ctx.enter_context(
            nc.sbuf_tensor("idxw", [P, ng // 16], mybir.dt.int16)
        )
        g2 = ctx.enter_context(
            nc.sbuf_tensor("g2", [P, TPP * DIM], mybir.dt.float32)
        )
        s_i32 = ctx.enter_context(nc.semaphore("s_i32"))
        s_iw = ctx.enter_context(nc.semaphore("s_iw"))
        s_gi = [ctx.enter_context(nc.semaphore(f"s_gi{j}")) for j in range(k)]
        s_gg = ctx.enter_context(nc.semaphore("s_gg"))
        s_out = ctx.enter_context(nc.semaphore("s_out"))

        nc.gpsimd.load_library(library_config.mlp)
        n_reg = nc.gpsimd.to_reg(ng)

        nc.sync.dma_start(idx32_t[:], x32[:]).then_inc(s_i32, 16)
        nc.sync.dma_start(idxw_t[:], xw[:]).then_inc(s_iw, 16)

        for it in range(loop_m):
            if it > 0:
                nc.sync.wait_ge(s_out, 32 * it)
            nc.gpsimd.wait_ge(s_i32, 16 * (it + 1))
            for j in range(k):
                nc.gpsimd.indirect_dma_start(
                    out=g2[:, j * DIM : (j + 1) * DIM],
                    out_offset=None,
                    in_=w[:],
                    in_offset=bass.IndirectOffsetOnAxis(
                        ap=idx32_t[:, j : j + 1], axis=0
                    ),
                ).then_inc(s_gi[j], 16)
            nc.gpsimd.wait_ge(s_iw, 16 * (it + 1))
            nc.gpsimd.dma_gather(
                g2[:, k * DIM :].rearrange("p (m d) -> p m d", d=DIM),
                w[:],
                idxw_t[:],
                ng,
                n_reg,
                DIM,
                queue_num=0,
            ).then_inc(s_gg, 16)
            for j in range(k):
                nc.sync.wait_ge(s_gi[j], 16 * (it + 1))
            nc.sync.dma_start(
                out[:, 0:k, :], g2[:, 0 : k * DIM]
            ).then_inc(s_out, 16)
            nc.scalar.wait_ge(s_gg, 16 * (it + 1))
            nc.scalar.dma_start(
                out[:, k:, :], g2[:, k * DIM :]
            ).then_inc(s_out, 16)
    nc.compile()
    return nc


def _wrap_idxs(tokens: np.ndarray) -> np.ndarray:
    """dma_gather idx layout: idx[p, s] = tokens[s*16 + p%16], [128, 64] i16."""
    t16 = tokens.reshape(IDX_COLS, 16).T.astype(np.int16)  # [16, 64]
    return np.ascontiguousarray(np.tile(t16, (P // 16, 1)))


_IOTA_WRAPPED = None


def _wrap_iota() -> np.ndarray:
    global _IOTA_WRAPPED
    if _IOTA_WRAPPED is None:
        _IOTA_WRAPPED = _wrap_idxs(np.arange(TPC, dtype=np.int64))
    return _IOTA_WRAPPED


def bench_in_maps(inputs):
    x_flat = inputs["x"].reshape(-1).astype(np.int64)
    w = np.ascontiguousarray(inputs["weight"].astype(np.float32))
    return [
        {
            "x": _wrap_idxs(x_flat[c * TPC : (c + 1) * TPC]),
            "weight": w,
            "wb_idx": _wrap_iota(),
        }
        for c in range(N_CORES)
    ]


def kernel(x: np.ndarray, weight: np.ndarray, **run_kwargs):
    global _NC_CACHE
    if _NC_CACHE is None:
        _NC_CACHE = build_nc_v6(
            shared_idx=True, shared_g=True, n_queues=1,
            wb_groups=((0, 1, 2, 3), (4, 5), (6,), (7,)),
            wb_engines=("sync", "act", "sync", "act"), lean=True,
        )
    nc = _NC_CACHE

    x_flat = np.asarray(x).reshape(-1).astype(np.int64)
    w = np.ascontiguousarray(np.asarray(weight, dtype=np.float32))

    in_maps = [
        {
            # shared_idx layout: x[p, j] = token p*TPP + j (see build_nc_v6)
            "x": np.ascontiguousarray(
                x_flat[c * TPC : (c + 1) * TPC].reshape(P, TPP).astype(np.int32)
            ),
            "weight": w,
        }
        for c in range(N_CORES)
    ]
    res = run_bass_kernel_spmd(nc, in_maps, core_ids=list(range(N_CORES)), **run_kwargs)
    # out [128, 8, 128]: token p*8+j lives at [p, j, :] -> plain reshape
    parts = [res.results[c]["out"].reshape(TPC, DIM) for c in range(N_CORES)]
    full = np.concatenate(parts, axis=0).reshape(B, S, DIM)
    if run_kwargs:
        return full, res
    return full


# revision 47
# speedup vs baseline: 1.0021x; 1.0021x over previous
"""Embedding lookup kernel for Trainium2 (8 NeuronCores, SPMD).

An embedding lookup IS a row gather: out[b, s, :] = weight[x[b, s], :].
Tokens are sharded 8 ways (1024 contiguous tokens per core); every core
keeps the full table in its DRAM. SHIPPED VARIANT (kernel() ->
build_nc_v6 shared_idx/shared_g/lean + staggered dual-engine writebacks):

Per core:
  1. One HWDGE DMA loads idx [128, 8] int32 into SBUF (idx[p, j] = token
     p*8 + j). Completion latency ~1.4-3us.
  2. A WARMUP indirect op (zeroed idx -> scratch) runs inside the idx
     latency window; it absorbs the first-op cold cost (~0.3-0.5us) so
     all real ops dispatch at the steady ~1.41us cadence.
  3. EIGHT indirect_dma_start ops (InstDMACopy + DynamicAccessPatternInfo
     on the qPoolDynamic SWDGE queue): op j gathers w[idx[p, j]] -> 512B
     row into g[p, j*128:(j+1)*128] for all 128 partitions. The HW
     indirect1d expansion consumes exactly ONE index per DEST PARTITION
     per op (one desc per index, elem = the partition's contiguous span),
     so 1024 rows need 8 ops; each occupies the Pool dispatch ~1.41us
     SERIALLY (SEQ<->Q7 command handshake; the queue attr on InstDMACopy
     does NOT route across SWDGE queues - only custom ucode instructions
     carry queue_num).
  4. Writebacks staggered over BOTH HWDGE engines - (0-3) sync, (4,5)
     act, (6) sync, (7) act - each fired the moment its last column's
     completion sem lands (lean waits: same-ring FIFO per SDMA engine
     means a group's last op's sem implies the whole group's data is in
     SBUF). The final writeback is a single 64KB column, minimizing the
     post-gather tail. All DMAs keep .then_inc (NEFF build requires a sem
     update per DMA).

One-shot cost on HW ~23.2-23.5us (fast device state): NEFF preamble ~5.9 (engine barriers +
TPB base loads + input-ready event; framework-emitted, not removable
from bass) + idx ~2.6 + 8x1.41 dispatch + last-op drain ~1.7 + final wb
~1.2 + epilogue ~1.1. Run-to-run variance +-1.5us (preamble/idx luck).

Measured dead ends (traces in this session):
- ONE batched indirect op ([P, TPP] idx, flat 2D dest) runs 15.2us total
  but gathers w[idx[p,0]+j] (contiguous run per partition) - wrong for
  arbitrary tokens. 3D dest APs scramble; DRAM->DRAM dest crashes the
  runtime.
- dma_gather (ucode): mlp LOAD_LIB costs 8.8us one-shot even hoisted
  first (v8 ~24.9); loop-amortized v4-line numbers do not transfer.
- v10 hybrid (indirect ops supposedly overlapping the lib load): the
  LOAD_LIB blocks even base-firmware SWDGE desc-gen until loaded (30.5us).
- ap_gather (vocab-sharded SBUF table): ~27ns/token at d=1 (41us);
  >=2MB DMAs concurrent with a library load starve the Q7 loader
  (2.3us -> 43us).
- ring_alt (PSEUDO_DMASWAP_QUEUE_SET between indirect ops to alternate
  SWDGE rings / Q7 cores): compiles, but the runtime crashes executing
  the NEFF - the swap pseudo is not accepted from raw-bass programs.
  (bass_rust codegen never emits it; InstDMACopy queue attr is cosmetic.)

loop_m > 1 builds the timing-harness variant (cross-iteration
serialization) used by bench scripts.
"""

import contextlib

import numpy as np

import concourse.bass as bass
from concourse import bacc, library_config, mybir
from concourse.bass_utils import run_bass_kernel_spmd

N_CORES = 8
B, S = 2, 4096
VOCAB, DIM = 32000, 128
P = 128
TOKENS = B * S                      # 8192
TPC = TOKENS // N_CORES             # 1024 tokens per core
TPP = TPC // P                      # 8 tokens per partition
IDX_COLS = TPC // 16                # 64 int16 idxs per partition row


def build_nc(loop_m: int = 1):
    # Skip the Bass-constructor entry barrier (gates the first DMA behind
    # all engines' init); restore the method right after construction.
    orig_barrier = bass.Bass.all_engine_barrier
    bass.Bass.all_engine_barrier = lambda self, *a, **k: None
    try:
        nc = bacc.Bacc(
            None, target_bir_lowering=False, dynamic_dma_scratch_size=32768
        )
    finally:
        bass.Bass.all_engine_barrier = orig_barrier

    x = nc.dram_tensor("x", [P, IDX_COLS], mybir.dt.int16, kind="ExternalInput")
    w = nc.dram_tensor("weight", [VOCAB, DIM], mybir.dt.float32, kind="ExternalInput")
    out = nc.dram_tensor("out", [P, TPP, DIM], mybir.dt.float32, kind="ExternalOutput")

    with contextlib.ExitStack() as ctx:
        idx_tile = ctx.enter_context(
            nc.sbuf_tensor("idx_tile", [P, IDX_COLS], mybir.dt.int16)
        )
        g = ctx.enter_context(nc.sbuf_tensor("g", [P, TPP, DIM], mybir.dt.float32))
        dummy_idx = ctx.enter_context(
            nc.sbuf_tensor("dummy_idx", [P, 8], mybir.dt.int16)
        )
        scratch = ctx.enter_context(
            nc.sbuf_tensor("scratch", [P, 1, DIM], mybir.dt.float32)
        )
        s_idx = ctx.enter_context(nc.semaphore("s_idx"))
        s_warm = ctx.enter_context(nc.semaphore("s_warm"))
        s_ms = ctx.enter_context(nc.semaphore("s_ms"))
        s_g = ctx.enter_context(nc.semaphore("s_g"))
        s_out = ctx.enter_context(nc.semaphore("s_out"))

        # Hoist the num_idxs register materialization off the critical path
        # (otherwise the mov lands after the s_idx wait).
        n_reg = nc.gpsimd.to_reg(TPC)

        # Warmup gather (128 zero indices), hidden inside the idx-DMA latency
        # window; also pulls the gpsimd library load off the critical path.
        nc.gpsimd.memset(dummy_idx[:], 0).then_inc(s_ms, 1)
        nc.gpsimd.wait_ge(s_ms, 1)
        nc.gpsimd.dma_gather(
            scratch[:], w[:], dummy_idx[:], P, P, DIM
        ).then_inc(s_warm, 16)

        # loop_m > 1 is the timing-harness mode: repeat the body with full
        # cross-iteration serialization (iter k+1's idx load waits for iter
        # k's writeback) so wall-time deltas measure per-iteration latency.
        for k in range(loop_m):
            if k > 0:
                nc.sync.wait_ge(s_out, 16 * k)
            nc.sync.dma_start(idx_tile[:], x[:]).then_inc(s_idx, 16)
            nc.gpsimd.wait_ge(s_idx, 16 * (k + 1))
            nc.gpsimd.dma_gather(
                g[:], w[:], idx_tile[:], TPC, n_reg, DIM
            ).then_inc(s_g, 16)
            nc.sync.wait_ge(s_g, 16 * (k + 1))
            nc.sync.dma_start(out[:], g[:]).then_inc(s_out, 16)
    nc.compile()
    return nc


def build_nc_v3(loop_m: int = 1):
    """v3: writeback via a prepared dma_scatter_add with iota indices.

    The scatter's descriptors (SBUF g -> DRAM out rows 0..1023) are generated
    on the Pool engine while the gather's data is still draining, then fired
    with trigger_dma as soon as the gather's completion semaphore arrives —
    removing the HWDGE dispatch from the tail. out rows are pre-zeroed by the
    runtime, so += is =. Output lands in natural token order [1024, 128].
    """
    orig_barrier = bass.Bass.all_engine_barrier
    bass.Bass.all_engine_barrier = lambda self, *a, **k: None
    try:
        nc = bacc.Bacc(
            None, target_bir_lowering=False, dynamic_dma_scratch_size=32768
        )
    finally:
        bass.Bass.all_engine_barrier = orig_barrier

    x = nc.dram_tensor("x", [P, IDX_COLS], mybir.dt.int16, kind="ExternalInput")
    wbx = nc.dram_tensor("wb_idx", [P, IDX_COLS], mybir.dt.int16, kind="ExternalInput")
    w = nc.dram_tensor("weight", [VOCAB, DIM], mybir.dt.float32, kind="ExternalInput")
    out = nc.dram_tensor("out", [TPC, DIM], mybir.dt.float32, kind="ExternalOutput")

    with contextlib.ExitStack() as ctx:
        idx_tile = ctx.enter_context(
            nc.sbuf_tensor("idx_tile", [P, IDX_COLS], mybir.dt.int16)
        )
        wbx_tile = ctx.enter_context(
            nc.sbuf_tensor("wbx_tile", [P, IDX_COLS], mybir.dt.int16)
        )
        g = ctx.enter_context(nc.sbuf_tensor("g", [P, TPP, DIM], mybir.dt.float32))
        dummy_idx = ctx.enter_context(
            nc.sbuf_tensor("dummy_idx", [P, 8], mybir.dt.int16)
        )
        scratch = ctx.enter_context(
            nc.sbuf_tensor("scratch", [P, 1, DIM], mybir.dt.float32)
        )
        s_idx = ctx.enter_context(nc.semaphore("s_idx"))
        s_wbx = ctx.enter_context(nc.semaphore("s_wbx"))
        s_warm = ctx.enter_context(nc.semaphore("s_warm"))
        s_ms = ctx.enter_context(nc.semaphore("s_ms"))
        s_g = ctx.enter_context(nc.semaphore("s_g"))
        s_wb = ctx.enter_context(nc.semaphore("s_wb"))
        s_prep = ctx.enter_context(nc.semaphore("s_prep"))

        nc.gpsimd.memset(dummy_idx[:], 0).then_inc(s_ms, 1)
        nc.gpsimd.wait_ge(s_ms, 1)
        nc.gpsimd.dma_gather(
            scratch[:], w[:], dummy_idx[:], P, P, DIM
        ).then_inc(s_warm, 16)

        nc.sync.dma_start(idx_tile[:], x[:]).then_inc(s_idx, 16)
        nc.sync.dma_start(wbx_tile[:], wbx[:]).then_inc(s_wbx, 16)

        for k in range(loop_m):
            if k > 0:
                nc.sync.wait_ge(s_wb, 16 * k)
                nc.sync.dma_start(idx_tile[:], x[:]).then_inc(s_idx, 16)
            nc.gpsimd.wait_ge(s_idx, 16 * (k + 1))
            nc.gpsimd.dma_gather(g[:], w[:], idx_tile[:], TPC, TPC, DIM).then_inc(
                s_g, 16
            )
            if k == 0:
                nc.gpsimd.wait_ge(s_wbx, 16)
            nc.gpsimd.dma_scatter_add(
                out[:], g[:], wbx_tile[:], TPC, TPC, DIM,
                prepare_only=True, sem=s_wb,
            ).then_inc(s_prep, 1)
            nc.gpsimd.wait_ge(s_prep, k + 1)
            nc.gpsimd.wait_ge(s_g, 16 * (k + 1))
            nc.gpsimd.trigger_dma(count=1)
        nc.gpsimd.wait_ge(s_wb, 16 * loop_m)
    nc.compile()
    return nc


def build_nc_v4(loop_m: int = 1, sizes=(512, 512), wb_engines=("sync",),
                n_queues: int = 1, warm_queues: int | None = None,
                warm_in_loop: bool = False, single_packet: bool = True,
                wb_groups=None):
    """v4: gather + writeback split into pipelined chunks of `sizes` tokens
    (each a multiple of 128). Chunk c's HWDGE writeback overlaps chunk c+1's
    gather desc-gen/drain, at the price of an extra ~1us SWDGE fixed overhead
    per extra chunk. wb_engines: round-robin engines for the writebacks
    ("sync" = SP, "act" = Activation). n_queues > 1 round-robins the gathers
    over that many SWDGE queues."""
    assert sum(sizes) == TPC and all(s % 128 == 0 for s in sizes)
    orig_barrier = bass.Bass.all_engine_barrier
    bass.Bass.all_engine_barrier = lambda self, *a, **k: None
    try:
        nc = bacc.Bacc(
            None, target_bir_lowering=False, dynamic_dma_scratch_size=32768,
            num_swdge_queues=n_queues, use_seq_codegen=seq_codegen,
        )
    finally:
        bass.Bass.all_engine_barrier = orig_barrier

    x = nc.dram_tensor("x", [P, IDX_COLS], mybir.dt.int16, kind="ExternalInput")
    w = nc.dram_tensor("weight", [VOCAB, DIM], mybir.dt.float32, kind="ExternalInput")
    out = nc.dram_tensor("out", [P, TPP, DIM], mybir.dt.float32, kind="ExternalOutput")

    chunks = len(sizes)
    bounds = [0]
    for s in sizes:
        bounds.append(bounds[-1] + s)

    with contextlib.ExitStack() as ctx:
        idx_tile = ctx.enter_context(
            nc.sbuf_tensor("idx_tile", [P, IDX_COLS], mybir.dt.int16)
        )
        g = ctx.enter_context(nc.sbuf_tensor("g", [P, TPP, DIM], mybir.dt.float32))
        dummy_idx = ctx.enter_context(
            nc.sbuf_tensor("dummy_idx", [P, 8], mybir.dt.int16)
        )
        scratch = ctx.enter_context(
            nc.sbuf_tensor("scratch", [P, max(n_queues, 1), DIM], mybir.dt.float32)
        )
        s_idx = ctx.enter_context(nc.semaphore("s_idx"))
        s_warms = [
            ctx.enter_context(nc.semaphore(f"s_warm{q}"))
            for q in range(max(warm_queues if warm_queues is not None else n_queues, 1))
        ]
        s_ms = ctx.enter_context(nc.semaphore("s_ms"))
        s_gs = [ctx.enter_context(nc.semaphore(f"s_g{c}")) for c in range(chunks)]
        s_out = ctx.enter_context(nc.semaphore("s_out"))

        n_regs = {}
        for s in dict.fromkeys(sizes):
            n_regs[s] = nc.gpsimd.to_reg(s)

        if warm_queues is None:
            # One warmup only: the q0 warmup absorbs the library load + ucode
            # cold cost. Extra per-queue warmups measured ~1.4us each and run
            # serially on the Pool engine, overrunning the ~2.3us idx-DMA
            # window — a guaranteed delay for an unproven per-queue saving.
            warm_queues = 1

        def emit_warmups():
            # One dummy gather per SWDGE queue: warms the ucode path, the
            # per-queue doorbell/ring state, and (queue 0, first) pulls the
            # library load off the critical path. Queue 0 gets the full
            # 128-idx warmup; the rest use 16 idxs (fixed cost dominates).
            for q in range(warm_queues):
                if q == 0:
                    nc.gpsimd.dma_gather(
                        scratch[:, 0:1, :], w[:], dummy_idx[:], P, P, DIM,
                        queue_num=0,
                    ).then_inc(s_warms[0], 16)
                else:
                    nc.gpsimd.dma_gather(
                        scratch[:, q : q + 1, :], w[:], dummy_idx[:, :1], 16, 16,
                        DIM, queue_num=q,
                    ).then_inc(s_warms[q], 16)

        nc.gpsimd.memset(dummy_idx[:], 0).then_inc(s_ms, 1)
        nc.gpsimd.wait_ge(s_ms, 1)
        emit_warmups()

        nc.sync.dma_start(idx_tile[:], x[:]).then_inc(s_idx, 16)

        engines = {"sync": nc.sync, "act": nc.scalar}

        n_wbs = len(wb_groups) if wb_groups else chunks
        for k in range(loop_m):
            if k > 0:
                nc.sync.wait_ge(s_out, 16 * n_wbs * k)
                nc.sync.dma_start(idx_tile[:], x[:]).then_inc(s_idx, 16)
            if warm_in_loop and k > 0:
                emit_warmups()
            nc.gpsimd.wait_ge(s_idx, 16 * (k + 1))
            for c in range(chunks):
                j0, j1 = bounds[c] // P, bounds[c + 1] // P
                nc.gpsimd.dma_gather(
                    g[:, j0:j1, :],
                    w[:],
                    idx_tile[:, bounds[c] // 16 : bounds[c + 1] // 16],
                    sizes[c],
                    n_regs[sizes[c]],
                    DIM,
                    queue_num=c % n_queues,
                    single_packet=single_packet,
                ).then_inc(s_gs[c], 16)
            groups = wb_groups or [(c,) for c in range(chunks)]
            for gi, grp in enumerate(groups):
                j0 = bounds[grp[0]] // P
                j1 = bounds[grp[-1] + 1] // P
                eng = engines[wb_engines[gi % len(wb_engines)]]
                for c in grp:
                    eng.wait_ge(s_gs[c], 16 * (k + 1))
                eng.dma_start(
                    out[:, j0:j1, :], g[:, j0:j1, :]
                ).then_inc(s_out, 16)
    nc.compile()
    return nc


def build_nc_v4b(loop_m: int = 1):
    return build_nc_v4(loop_m, sizes=(640, 384))


def build_nc_v4c(loop_m: int = 1):
    return build_nc_v4(loop_m, sizes=(512, 256, 256))


def build_nc_v4d(loop_m: int = 1):
    return build_nc_v4(loop_m, sizes=(256, 256, 256, 256))


def build_nc_v4c2(loop_m: int = 1):
    return build_nc_v4(loop_m, sizes=(512, 256, 256), wb_engines=("sync", "act"))


def build_nc_v4asc(loop_m: int = 1):
    return build_nc_v4(loop_m, sizes=(256, 256, 512))


def build_nc_v4ascq(loop_m: int = 1):
    return build_nc_v4(loop_m, sizes=(256, 256, 512), n_queues=3)


def build_nc_v4ascq_w(loop_m: int = 1):
    # probe: per-queue warmups re-run inside every loop iteration
    return build_nc_v4(loop_m, sizes=(256, 256, 512), n_queues=3,
                       warm_in_loop=True)


def build_nc_v4ascq2(loop_m: int = 1):
    return build_nc_v4(loop_m, sizes=(256, 256, 512), n_queues=2)


def build_nc_v4eq(loop_m: int = 1):
    return build_nc_v4(loop_m, sizes=(128, 128, 256, 512), n_queues=4)


def build_nc_v4gq(loop_m: int = 1):
    return build_nc_v4(loop_m, sizes=(128, 128, 128, 128, 512), n_queues=4)


def _probe_builder(loop_m: int, *, no_idx: bool = False, no_wb: bool = False,
                   sizes=(128, 128, 256, 512), n_queues: int = 4):
    """Timing probes: v4eq with the per-iteration idx DMA and/or the
    writebacks removed, to decompose per-iteration time on HW."""
    orig_barrier = bass.Bass.all_engine_barrier
    bass.Bass.all_engine_barrier = lambda self, *a, **k: None
    try:
        nc = bacc.Bacc(
            None, target_bir_lowering=False, dynamic_dma_scratch_size=32768,
            num_swdge_queues=n_queues, use_seq_codegen=seq_codegen,
        )
    finally:
        bass.Bass.all_engine_barrier = orig_barrier

    x = nc.dram_tensor("x", [P, IDX_COLS], mybir.dt.int16, kind="ExternalInput")
    w = nc.dram_tensor("weight", [VOCAB, DIM], mybir.dt.float32, kind="ExternalInput")
    out = nc.dram_tensor("out", [P, TPP, DIM], mybir.dt.float32, kind="ExternalOutput")

    chunks = len(sizes)
    bounds = [0]
    for s in sizes:
        bounds.append(bounds[-1] + s)

    with contextlib.ExitStack() as ctx:
        idx_tile = ctx.enter_context(
            nc.sbuf_tensor("idx_tile", [P, IDX_COLS], mybir.dt.int16)
        )
        g = ctx.enter_context(nc.sbuf_tensor("g", [P, TPP, DIM], mybir.dt.float32))
        dummy_idx = ctx.enter_context(
            nc.sbuf_tensor("dummy_idx", [P, 8], mybir.dt.int16)
        )
        scratch = ctx.enter_context(
            nc.sbuf_tensor("scratch", [P, 1, DIM], mybir.dt.float32)
        )
        s_idx = ctx.enter_context(nc.semaphore("s_idx"))
        s_warm = ctx.enter_context(nc.semaphore("s_warm"))
        s_ms = ctx.enter_context(nc.semaphore("s_ms"))
        s_gs = [ctx.enter_context(nc.semaphore(f"s_g{c}")) for c in range(chunks)]
        s_out = ctx.enter_context(nc.semaphore("s_out"))

        n_regs = {}
        for s in dict.fromkeys(sizes):
            n_regs[s] = nc.gpsimd.to_reg(s)

        nc.gpsimd.memset(dummy_idx[:], 0).then_inc(s_ms, 1)
        nc.gpsimd.wait_ge(s_ms, 1)
        nc.gpsimd.dma_gather(
            scratch[:], w[:], dummy_idx[:], P, P, DIM, queue_num=0
        ).then_inc(s_warm, 16)

        nc.sync.dma_start(idx_tile[:], x[:]).then_inc(s_idx, 16)

        for k in range(loop_m):
            if no_idx:
                if k > 0:
                    # serialize iterations + WAR-protect g without an idx DMA
                    nc.gpsimd.wait_ge(
                        s_out if not no_wb else s_gs[-1],
                        (16 * chunks * k) if not no_wb else 16 * k,
                    )
                nc.gpsimd.wait_ge(s_idx, 16)
            else:
                if k > 0:
                    if no_wb:
                        for c in range(chunks):
                            nc.sync.wait_ge(s_gs[c], 16 * k)
                    else:
                        nc.sync.wait_ge(s_out, 16 * chunks * k)
                    nc.sync.dma_start(idx_tile[:], x[:]).then_inc(s_idx, 16)
                nc.gpsimd.wait_ge(s_idx, 16 * (k + 1))
            for c in range(chunks):
                j0, j1 = bounds[c] // P, bounds[c + 1] // P
                nc.gpsimd.dma_gather(
                    g[:, j0:j1, :],
                    w[:],
                    idx_tile[:, bounds[c] // 16 : bounds[c + 1] // 16],
                    sizes[c],
                    n_regs[sizes[c]],
                    DIM,
                    queue_num=c % n_queues,
                ).then_inc(s_gs[c], 16)
            if not no_wb:
                for c in range(chunks):
                    j0, j1 = bounds[c] // P, bounds[c + 1] // P
                    nc.sync.wait_ge(s_gs[c], 16 * (k + 1))
                    nc.sync.dma_start(
                        out[:, j0:j1, :], g[:, j0:j1, :]
                    ).then_inc(s_out, 16)
        if no_wb:
            # write the output once at the end so the ExternalOutput has a
            # writer (walrus requires semaphore updates per DMA anyway)
            for c in range(chunks):
                nc.sync.wait_ge(s_gs[c], 16 * loop_m)
            nc.sync.dma_start(out[:], g[:]).then_inc(s_out, 16)
    nc.compile()
    return nc


def build_nc_v4mq(loop_m: int = 1):
    # merged writeback for the two 128-token primer chunks
    return build_nc_v4(loop_m, sizes=(128, 128, 256, 512), n_queues=4,
                       wb_groups=[(0, 1), (2,), (3,)])


def build_nc_v4sp(loop_m: int = 1):
    return build_nc_v4(loop_m, sizes=(128, 128, 256, 512), n_queues=4,
                       single_packet=False)


def build_nc_v4hq(loop_m: int = 1):
    # tail-light ordering: big chunk third, tiny chunk last so the final
    # drain + writeback are short
    return build_nc_v4(loop_m, sizes=(128, 256, 512, 128), n_queues=4)


def build_nc_v4eq_noidx(loop_m: int = 1):
    return _probe_builder(loop_m, no_idx=True)


def build_nc_v4eq_nowb(loop_m: int = 1):
    return _probe_builder(loop_m, no_wb=True)


def build_nc_v4eq_gonly(loop_m: int = 1):
    return _probe_builder(loop_m, no_idx=True, no_wb=True)


def build_nc_v4e(loop_m: int = 1):
    return build_nc_v4(loop_m, sizes=(128, 128, 256, 512))


def build_nc_v4cq(loop_m: int = 1):
    return build_nc_v4(loop_m, sizes=(512, 256, 256), n_queues=3)


def build_nc_v4q2(loop_m: int = 1):
    return build_nc_v4(loop_m, sizes=(512, 512), n_queues=2)


def build_nc_v4dq(loop_m: int = 1):
    return build_nc_v4(loop_m, sizes=(256, 256, 256, 256), n_queues=4)


def build_nc_v5(loop_m: int = 1, sizes=(TPP,), wb_groups=None):
    """v5: ONE batched vector-indirect DMA (InstDMACopy on qPoolDynamic)
    instead of gpsimd dma_gather ucode.

    dma_gather needs the `mlp` gpsimd library; its LOAD_LIB DMA (~8.8us) +
    warmup gather (~1.6us) sit on the one-shot critical path (the graded
    metric is a single NEFF execution, so the loop-harness amortization the
    v4 line was tuned for never happens). indirect_dma_start lowers to a
    plain DMA with DynamicAccessPatternInfo - descriptor expansion happens
    in the SWDGE base path, no library load, no ucode warmup.

    Index layout: idx_tile[p, j] = token p*TPP + j (int32, [128, 8]); each
    index gathers one contiguous 512B row of w; row i (offset-AP C-order
    p*TPP+j) lands at output block i = g[p, j, :]. Host reshape(TPC, DIM)
    restores token order. `sizes` splits the gather into chunks of TPP
    columns each so chunk c's HWDGE writeback overlaps chunk c+1's drain.
    """
    assert sum(sizes) == TPP
    orig_barrier = bass.Bass.all_engine_barrier
    bass.Bass.all_engine_barrier = lambda self, *a, **k: None
    try:
        nc = bacc.Bacc(
            None, target_bir_lowering=False, dynamic_dma_scratch_size=32768
        )
    finally:
        bass.Bass.all_engine_barrier = orig_barrier

    x = nc.dram_tensor("x", [P, TPP], mybir.dt.int32, kind="ExternalInput")
    w = nc.dram_tensor("weight", [VOCAB, DIM], mybir.dt.float32, kind="ExternalInput")
    out = nc.dram_tensor("out", [P, TPP, DIM], mybir.dt.float32, kind="ExternalOutput")

    chunks = len(sizes)
    bounds = [0]
    for s in sizes:
        bounds.append(bounds[-1] + s)

    with contextlib.ExitStack() as ctx:
        idx_tile = ctx.enter_context(
            nc.sbuf_tensor("idx_tile", [P, TPP], mybir.dt.int32)
        )
        g = ctx.enter_context(nc.sbuf_tensor("g", [P, TPP, DIM], mybir.dt.float32))
        s_idx = ctx.enter_context(nc.semaphore("s_idx"))
        s_gs = [ctx.enter_context(nc.semaphore(f"s_g{c}")) for c in range(chunks)]
        s_out = ctx.enter_context(nc.semaphore("s_out"))

        groups = wb_groups or [(c,) for c in range(chunks)]
        n_wbs = len(groups)
        for k in range(loop_m):
            if k > 0:
                nc.sync.wait_ge(s_out, 16 * n_wbs * k)
            nc.sync.dma_start(idx_tile[:], x[:]).then_inc(s_idx, 16)
            nc.gpsimd.wait_ge(s_idx, 16 * (k + 1))
            for c in range(chunks):
                j0, j1 = bounds[c], bounds[c + 1]
                nc.gpsimd.indirect_dma_start(
                    out=g[:, j0:j1, :],
                    out_offset=None,
                    in_=w[:],
                    in_offset=bass.IndirectOffsetOnAxis(
                        ap=idx_tile[:, j0:j1], axis=0
                    ),
                ).then_inc(s_gs[c], 16)
            for grp in groups:
                j0, j1 = bounds[grp[0]], bounds[grp[-1] + 1]
                for c in grp:
                    nc.sync.wait_ge(s_gs[c], 16 * (k + 1))
                nc.sync.dma_start(
                    out[:, j0:j1, :], g[:, j0:j1, :]
                ).then_inc(s_out, 16)
    nc.compile()
    return nc


IDXW = 64  # idx tile words/partition; 256B pitch is required for the
           # batched vector-indirect op to fetch one index per dest block
           # (32B pitch degrades to one index per partition + contiguous run)


def build_nc_v7(loop_m: int = 1, wb_groups=((0, TPP),)):
    """v7: ONE batched vector-indirect DMA for all 1024 rows.

    Probed on HW (dbg_walk.py): with the idx tile at 256B partition pitch
    ([P, 64] int32, first TPP words used) and a flat 2D dest [P, TPP*DIM],
    the SWDGE indirect1d expansion consumes exactly one index per 512B dest
    block in (p, j) C-order: g[p, j*DIM:(j+1)*DIM] = w[idx[p, j]]. One Pool
    dispatch (~1.1us) replaces 8 (~11.3us serial).

    wb_groups: list of (j0, j1) column ranges, one HWDGE writeback each,
    gather split per group so wb overlaps the later groups' drains.
    """
    orig_barrier = bass.Bass.all_engine_barrier
    bass.Bass.all_engine_barrier = lambda self, *a, **k: None
    try:
        nc = bacc.Bacc(
            None, target_bir_lowering=False, dynamic_dma_scratch_size=32768
        )
    finally:
        bass.Bass.all_engine_barrier = orig_barrier

    x = nc.dram_tensor("x", [P, IDXW], mybir.dt.int32, kind="ExternalInput")
    w = nc.dram_tensor("weight", [VOCAB, DIM], mybir.dt.float32, kind="ExternalInput")
    out = nc.dram_tensor("out", [P, TPP, DIM], mybir.dt.float32, kind="ExternalOutput")

    with contextlib.ExitStack() as ctx:
        idx_t = ctx.enter_context(nc.sbuf_tensor("idx", [P, IDXW], mybir.dt.int32))
        g_t = ctx.enter_context(
            nc.sbuf_tensor("g", [P, TPP * DIM], mybir.dt.float32)
        )
        s_idx = ctx.enter_context(nc.semaphore("s_idx"))
        s_gs = [ctx.enter_context(nc.semaphore(f"s_g{i}")) for i in range(len(wb_groups))]
        s_out = ctx.enter_context(nc.semaphore("s_out"))

        for k in range(loop_m):
            if k > 0:
                nc.sync.wait_ge(s_out, 16 * len(wb_groups) * k)
            nc.sync.dma_start(idx_t[:], x[:]).then_inc(s_idx, 16)
            nc.gpsimd.wait_ge(s_idx, 16 * (k + 1))
            for gi, (j0, j1) in enumerate(wb_groups):
                nc.gpsimd.indirect_dma_start(
                    out=g_t[:, j0 * DIM : j1 * DIM],
                    out_offset=None,
                    in_=w[:],
                    in_offset=bass.IndirectOffsetOnAxis(
                        ap=idx_t[:, j0:j1], axis=0
                    ),
                ).then_inc(s_gs[gi], 16)
            for gi, (j0, j1) in enumerate(wb_groups):
                nc.sync.wait_ge(s_gs[gi], 16 * (k + 1))
                nc.sync.dma_start(
                    out[:, j0:j1, :], g_t[:, j0 * DIM : j1 * DIM]
                ).then_inc(s_out, 16)
    nc.compile()
    return nc


def build_nc_v6(loop_m: int = 1, n_queues: int = 1,
                wb_groups=((0, 1, 2, 3), (4, 5, 6, 7)),
                col_order=None, shared_idx: bool = False,
                shared_g: bool = False, batched: bool = False,
                wb_engines=("sync",), lean: bool = False,
                ring_alt: bool = False):
    """v6: 8 per-column vector-indirect DMAs (the HW-proven expansion shape:
    ONE index per partition per op - idx[:, j:j+1] -> g[:, j, :]).

    No gpsimd ucode library, no LOAD_LIB (~8.8us) and no warmup (~1.6us) on
    the one-shot critical path. `n_queues` > 1 round-robins the ops across
    qPoolDynamic{i} SWDGE queues (desc-gen parallelizes across queues; the
    InstDMACopy queue attr is patched post-construction). wb_groups: column
    groups per HWDGE writeback, fired as soon as member gathers complete.
    """
    orig_barrier = bass.Bass.all_engine_barrier
    bass.Bass.all_engine_barrier = lambda self, *a, **k: None
    try:
        nc = bacc.Bacc(
            None, target_bir_lowering=False, dynamic_dma_scratch_size=32768,
            num_swdge_queues=n_queues,
        )
    finally:
        bass.Bass.all_engine_barrier = orig_barrier

    # shared_idx: x[p, j] = token p*TPP + j (one contiguous [P, TPP] load).
    # else: x[j, p] = token p*TPP + j so each column-op's [P, 1] idx load is
    # a contiguous 512B DRAM read.
    x_shape = [P, TPP] if shared_idx else [TPP, P]
    x = nc.dram_tensor("x", x_shape, mybir.dt.int32, kind="ExternalInput")
    w = nc.dram_tensor("weight", [VOCAB, DIM], mybir.dt.float32, kind="ExternalInput")
    out = nc.dram_tensor("out", [P, TPP, DIM], mybir.dt.float32, kind="ExternalOutput")

    cols = col_order or list(range(TPP))

    with contextlib.ExitStack() as ctx:
        # Baseline HW-proven shape (tile_scatter_add): per-op dest tile
        # [P, DIM] 2D at offset 0, per-op idx tile [P, 1] 2D at offset 0.
        # shared_idx / shared_g probe whether sliced (offset != 0) 2D APs
        # also work, which collapses the 8 idx loads / 8 writebacks.
        if shared_idx:
            idx_t = ctx.enter_context(
                nc.sbuf_tensor("idx", [P, TPP], mybir.dt.int32)
            )
            idx_aps = [idx_t[:, j : j + 1] for j in range(TPP)]
        else:
            idx_tiles = [
                ctx.enter_context(
                    nc.sbuf_tensor(f"idx{j}", [P, 1], mybir.dt.int32)
                )
                for j in range(TPP)
            ]
            idx_aps = [t[:] for t in idx_tiles]
        if shared_g:
            g_t = ctx.enter_context(
                nc.sbuf_tensor("g", [P, TPP * DIM], mybir.dt.float32)
            )
            g_aps = [g_t[:, j * DIM : (j + 1) * DIM] for j in range(TPP)]
        else:
            g_tiles = [
                ctx.enter_context(
                    nc.sbuf_tensor(f"g{j}", [P, DIM], mybir.dt.float32)
                )
                for j in range(TPP)
            ]
            g_aps = [t[:] for t in g_tiles]
        s_idx = ctx.enter_context(nc.semaphore("s_idx"))
        s_gs = [ctx.enter_context(nc.semaphore(f"s_g{j}")) for j in range(TPP)]
        s_out = ctx.enter_context(nc.semaphore("s_out"))
        if lean:
            warm_idx = ctx.enter_context(
                nc.sbuf_tensor("warm_idx", [P, 1], mybir.dt.int32)
            )
            warm_g = ctx.enter_context(
                nc.sbuf_tensor("warm_g", [P, DIM], mybir.dt.float32)
            )
            s_wm = ctx.enter_context(nc.semaphore("s_wm"))

        n_wbs = len(wb_groups)
        for k in range(loop_m):
            if k > 0:
                nc.sync.wait_ge(s_out, 16 * n_wbs * k)
            if shared_idx:
                nc.sync.dma_start(idx_t[:], x[:]).then_inc(s_idx, 16)
                idx_target = 16
            else:
                for j in range(TPP):
                    nc.sync.dma_start(
                        idx_aps[j], x[j : j + 1, :].rearrange("a b -> b a")
                    ).then_inc(s_idx, 16)
                idx_target = 16 * TPP
            if lean and k == 0:
                # warmup indirect op in the idle idx-latency window: zeroed
                # idx tile -> gathers w[0] into scratch; absorbs the first-op
                # cold cost off the critical path (dispatched before the idx
                # wait, ~6.0-7.4us while idx is still in flight)
                nc.gpsimd.memset(warm_idx[:], 0).then_inc(s_wm, 1)
                nc.gpsimd.wait_ge(s_wm, 1)
                nc.gpsimd.indirect_dma_start(
                    out=warm_g[:],
                    out_offset=None,
                    in_=w[:],
                    in_offset=bass.IndirectOffsetOnAxis(
                        ap=warm_idx[:], axis=0
                    ),
                ).then_inc(s_wm, 16)
            nc.gpsimd.wait_ge(s_idx, idx_target * (k + 1))
            if batched:
                # ONE 2D-everything op: dest [P, TPP*DIM] iterates TPP
                # 512B blocks per partition in lockstep with idx [P, TPP]
                assert shared_idx and shared_g
                for gi, grp in enumerate(wb_groups):
                    j0, j1 = grp[0], grp[-1] + 1
                    inst = nc.gpsimd.indirect_dma_start(
                        out=g_t[:, j0 * DIM : j1 * DIM],
                        out_offset=None,
                        in_=w[:],
                        in_offset=bass.IndirectOffsetOnAxis(
                            ap=idx_t[:, j0:j1], axis=0
                        ),
                    )
                    q = gi % n_queues
                    if q:
                        inst.ins.queue = f"qPoolDynamic{q}"
                    inst.then_inc(s_gs[grp[0]], 16)
            else:
                # every DMA needs a sem update (NEFF build requirement);
                # lean-ness lives on the wait side only
                sem_cols = set(cols)
                cur_ring = [0]

                def swap_ring(q):
                    if q == cur_ring[0]:
                        return
                    cur_ring[0] = q
                    qname = f"qPoolDynamic{q or ''}"
                    nc.gpsimd.isa(
                        nc.isa.Opcode.NEURON_ISA_TPB_OPCODE_PSEUDO_DMASWAP_QUEUE_SET,
                        {
                            "queue_instance_name": list(
                                qname.encode().ljust(32, b"\x00")
                            ),
                            "no_rearm": 0,
                            "reset_semaphores": 0,
                        },
                        struct_name="NEURON_ISA_TPB_PSEUDO_DMA_SWAP_QUEUE_SET_STRUCT",
                        verify=False,
                    )

                for i, j in enumerate(cols):
                    if ring_alt:
                        swap_ring(i % n_queues)
                    inst = nc.gpsimd.indirect_dma_start(
                        out=g_aps[j],
                        out_offset=None,
                        in_=w[:],
                        in_offset=bass.IndirectOffsetOnAxis(
                            ap=idx_aps[j], axis=0
                        ),
                    )
                    q = i % n_queues
                    if q:
                        inst.ins.queue = f"qPoolDynamic{q}"
                    if j in sem_cols:
                        inst.then_inc(s_gs[j], 16)
                if ring_alt:
                    swap_ring(0)
            engs = {"sync": nc.sync, "act": nc.scalar}
            for gi, grp in enumerate(wb_groups):
                eng = engs[wb_engines[gi % len(wb_engines)]]
                # lean: ops share one SWDGE ring; per-SDMA-engine FIFO means
                # the group's LAST op completing implies all earlier ops'
                # data landed, so waiting its sem alone is sound
                wait_cols = (
                    grp[:1]
                    if batched
                    else (grp[-1:] if (lean and not ring_alt) else grp)
                )
                for j in wait_cols:
                    eng.wait_ge(s_gs[j], 16 * (k + 1))
                if shared_g:
                    j0, j1 = grp[0], grp[-1] + 1
                    eng.dma_start(
                        out[:, j0:j1, :], g_t[:, j0 * DIM : j1 * DIM]
                    ).then_inc(s_out, 16)
                else:
                    for j in grp:
                        eng.dma_start(
                            out[:, j : j + 1, :], g_aps[j]
                        ).then_inc(s_out, 16)
    nc.compile()
    return nc


def build_nc_v8(loop_m: int = 1, sizes=(128, 128, 256, 512), n_queues=4,
                wb_groups=((0, 1), (2,), (3,)), warmup=False,
                seq_codegen=False):
    """v8: one-shot-optimized dma_gather. The mlp ucode library load
    (~8.8us) is dispatched as the FIRST Pool instruction (explicit
    load_library) so it fully overlaps the idx DMA + its latency; no
    warmup gather (its ~1.6us serial cost is not worth it one-shot - the
    first real gather pays the cold cost while later chunks' desc-gen
    queues behind it anyway). Chunked gathers on separate SWDGE queues
    (v4 lesson: dispatch is ~70ns when a Q7 core is free; chunks must be
    pow2 sizes; ascending order primes writeback earliest)."""
    assert sum(sizes) == TPC and all(s % 128 == 0 for s in sizes)
    orig_barrier = bass.Bass.all_engine_barrier
    bass.Bass.all_engine_barrier = lambda self, *a, **k: None
    try:
        nc = bacc.Bacc(
            None, target_bir_lowering=False, dynamic_dma_scratch_size=32768,
            num_swdge_queues=n_queues, use_seq_codegen=seq_codegen,
        )
    finally:
        bass.Bass.all_engine_barrier = orig_barrier

    x = nc.dram_tensor("x", [P, IDX_COLS], mybir.dt.int16, kind="ExternalInput")
    w = nc.dram_tensor("weight", [VOCAB, DIM], mybir.dt.float32, kind="ExternalInput")
    out = nc.dram_tensor("out", [P, TPP, DIM], mybir.dt.float32, kind="ExternalOutput")

    chunks = len(sizes)
    bounds = [0]
    for s in sizes:
        bounds.append(bounds[-1] + s)

    with contextlib.ExitStack() as ctx:
        idx_tile = ctx.enter_context(
            nc.sbuf_tensor("idx_tile", [P, IDX_COLS], mybir.dt.int16)
        )
        g = ctx.enter_context(nc.sbuf_tensor("g", [P, TPP, DIM], mybir.dt.float32))
        dummy_idx = ctx.enter_context(
            nc.sbuf_tensor("dummy_idx", [P, 8], mybir.dt.int16)
        )
        scratch = ctx.enter_context(
            nc.sbuf_tensor("scratch", [P, 1, DIM], mybir.dt.float32)
        )
        s_idx = ctx.enter_context(nc.semaphore("s_idx"))
        s_ms = ctx.enter_context(nc.semaphore("s_ms"))
        s_warm = ctx.enter_context(nc.semaphore("s_warm"))
        s_gs = [ctx.enter_context(nc.semaphore(f"s_g{c}")) for c in range(chunks)]
        s_out = ctx.enter_context(nc.semaphore("s_out"))

        # start the ucode library DMA immediately; it runs while the idx
        # DMA + HBM latency elapse
        nc.gpsimd.load_library(library_config.mlp)

        n_regs = {}
        for s in dict.fromkeys(sizes):
            n_regs[s] = nc.gpsimd.to_reg(s)

        if warmup:
            nc.gpsimd.memset(dummy_idx[:], 0).then_inc(s_ms, 1)
            nc.gpsimd.wait_ge(s_ms, 1)
            nc.gpsimd.dma_gather(
                scratch[:], w[:], dummy_idx[:], P, P, DIM, queue_num=0
            ).then_inc(s_warm, 16)

        nc.sync.dma_start(idx_tile[:], x[:]).then_inc(s_idx, 16)

        n_wbs = len(wb_groups)
        for k in range(loop_m):
            if k > 0:
                nc.sync.wait_ge(s_out, 16 * n_wbs * k)
                nc.sync.dma_start(idx_tile[:], x[:]).then_inc(s_idx, 16)
            nc.gpsimd.wait_ge(s_idx, 16 * (k + 1))
            for c in range(chunks):
                j0, j1 = bounds[c] // P, bounds[c + 1] // P
                nc.gpsimd.dma_gather(
                    g[:, j0:j1, :],
                    w[:],
                    idx_tile[:, bounds[c] // 16 : bounds[c + 1] // 16],
                    sizes[c],
                    n_regs[sizes[c]],
                    DIM,
                    queue_num=c % n_queues,
                ).then_inc(s_gs[c], 16)
            for grp in wb_groups:
                j0 = bounds[grp[0]] // P
                j1 = bounds[grp[-1] + 1] // P
                for c in grp:
                    nc.sync.wait_ge(s_gs[c], 16 * (k + 1))
                nc.sync.dma_start(
                    out[:, j0:j1, :], g[:, j0:j1, :]
                ).then_inc(s_out, 16)
    nc.compile()
    return nc


VSH = VOCAB // N_CORES   # 4000 vocab rows per core (vocab-sharded table)
NTOK = 1536              # padded per-core token capacity (exp ~1024, 8.5+sigma)


def build_nc_v9(loop_m: int = 1, n_chunks: int = 1):
    """v9: vocab-sharded ap_gather (SBUF-resident transposed table shard).

    Core c owns vocab rows [c*VSH, (c+1)*VSH); host routes each token to its
    owning core (the sharding_hint's vocab-parallel scheme) and un-permutes
    on the way out. Pipeline per core:
      1. load_library(ap_gather) - small lib, ~2.3us clean
      2. tiny warmup ap_gather; its completion sem doubles as the only
         observable "library loaded" signal, gating the big shard DMA (a
         2MB DMA concurrent with the Q7 library load starves the loader:
         measured 43us vs 2.3us)
      3. WT shard [128 dim, VSH] fp32 DMA -> SBUF (~6.5us at 313GB/s)
      4. one ap_gather: g[dim_p, k] = WT[dim_p, loc_idx[k]], all NTOK tokens
      5. HWDGE writeback g [128, NTOK] -> out DRAM (host transposes back)
    n_chunks > 1 splits 3-5 into vocab-range chunks (host buckets tokens
    per chunk) so shard-DMA / gather / writeback pipeline.
    """
    orig_barrier = bass.Bass.all_engine_barrier
    bass.Bass.all_engine_barrier = lambda self, *a, **k: None
    try:
        nc = bacc.Bacc(
            None, target_bir_lowering=False, dynamic_dma_scratch_size=32768
        )
    finally:
        bass.Bass.all_engine_barrier = orig_barrier

    assert VSH % n_chunks == 0 and NTOK % n_chunks == 0
    vch = VSH // n_chunks
    tch = NTOK // n_chunks

    wt = nc.dram_tensor("wt", [P, VSH], mybir.dt.float32, kind="ExternalInput")
    xw = nc.dram_tensor("xw", [P, NTOK // 16], mybir.dt.int16, kind="ExternalInput")
    out = nc.dram_tensor("out", [P, NTOK], mybir.dt.float32, kind="ExternalOutput")

    with contextlib.ExitStack() as ctx:
        wt_t = ctx.enter_context(nc.sbuf_tensor("wt_t", [P, VSH, 1], mybir.dt.float32))
        idx_t = ctx.enter_context(
            nc.sbuf_tensor("idx_t", [P, NTOK // 16], mybir.dt.int16)
        )
        g_t = ctx.enter_context(nc.sbuf_tensor("g_t", [P, NTOK, 1], mybir.dt.float32))
        dummy = ctx.enter_context(nc.sbuf_tensor("dmy_idx", [P, 1], mybir.dt.int16))
        scr = ctx.enter_context(nc.sbuf_tensor("scr", [P, 16, 1], mybir.dt.float32))
        s_ms = ctx.enter_context(nc.semaphore("s_ms"))
        s_lib = ctx.enter_context(nc.semaphore("s_lib"))
        s_idx = ctx.enter_context(nc.semaphore("s_idx"))
        s_wts = [ctx.enter_context(nc.semaphore(f"s_wt{c}")) for c in range(n_chunks)]
        s_gs = [ctx.enter_context(nc.semaphore(f"s_g{c}")) for c in range(n_chunks)]
        s_out = ctx.enter_context(nc.semaphore("s_out"))

        nc.gpsimd.load_library(library_config.ap_gather)
        # lib-loaded gate: first custom op blocks until the library lands
        nc.gpsimd.memset(dummy[:], 0).then_inc(s_ms, 1)
        nc.gpsimd.wait_ge(s_ms, 1)
        nc.gpsimd.ap_gather(
            scr[:], wt_t[:, :16, :], dummy[:], channels=P, num_elems=16, d=1,
            num_idxs=16,
        ).then_inc(s_lib, 1)

        # idx DMA is tiny - safe concurrent with the library load
        nc.sync.dma_start(idx_t[:], xw[:]).then_inc(s_idx, 16)
        nc.sync.wait_ge(s_lib, 1)

        for k in range(loop_m):
            if k > 0:
                nc.sync.wait_ge(s_out, 16 * n_chunks * k)
            for c in range(n_chunks):
                nc.sync.dma_start(
                    wt_t[:, c * vch : (c + 1) * vch, 0],
                    wt[:, c * vch : (c + 1) * vch],
                ).then_inc(s_wts[c], 16)
            nc.gpsimd.wait_ge(s_idx, 16)
            for c in range(n_chunks):
                nc.gpsimd.wait_ge(s_wts[c], 16 * (k + 1))
                nc.gpsimd.ap_gather(
                    g_t[:, c * tch : (c + 1) * tch, :],
                    wt_t[:, c * vch : (c + 1) * vch, :],
                    idx_t[:, c * (tch // 16) : (c + 1) * (tch // 16)],
                    channels=P,
                    num_elems=vch,
                    d=1,
                    num_idxs=tch,
                ).then_inc(s_gs[c], 1)
            for c in range(n_chunks):
                nc.sync.wait_ge(s_gs[c], k + 1)
                nc.sync.dma_start(
                    out[:, c * tch : (c + 1) * tch],
                    g_t[:, c * tch : (c + 1) * tch, 0],
                ).then_inc(s_out, 16)
    nc.compile()
    return nc


def _v9_in_maps(x_flat: np.ndarray, w: np.ndarray):
    """Route tokens to vocab-owning cores; return in_maps + unpermute info."""
    owner = x_flat // VSH                       # owning core per token
    order = np.argsort(owner, kind="stable")    # token positions grouped by core
    counts = np.bincount(owner, minlength=N_CORES)
    assert counts.max() <= NTOK, f"token bucket overflow: {counts.max()} > {NTOK}"
    in_maps = []
    for c in range(N_CORES):
        sel = order[counts[:c].sum() : counts[: c + 1].sum()]
        loc = (x_flat[sel] - c * VSH).astype(np.int16)
        locp = np.zeros(NTOK, dtype=np.int16)
        locp[: len(sel)] = loc
        t16 = locp.reshape(NTOK // 16, 16).T    # wrap for gpsimd stripes
        in_maps.append({
            "wt": np.ascontiguousarray(w[c * VSH : (c + 1) * VSH, :].T),
            "xw": np.ascontiguousarray(np.tile(t16, (P // 16, 1))),
        })
    return in_maps, order, counts


_NC_CACHE = None


def _wrap_idxs_n(tokens: np.ndarray) -> np.ndarray:
    """dma_gather idx wrap for arbitrary n%16==0 token lists."""
    n = len(tokens)
    t16 = tokens.reshape(n // 16, 16).T.astype(np.int16)
    return np.ascontiguousarray(np.tile(t16, (P // 16, 1)))


def build_nc_v10(loop_m: int = 1, k: int = 4):
    """v10 hybrid: overlap the two gather mechanisms' serial setups.

    Columns 0..k-1 (k*128 tokens) go through library-free vector-indirect
    ops whose ~1.41us/op serial dispatches start as soon as the idx DMA
    lands (~8.6us) - WHILE the mlp ucode library (~8.8us) loads in the
    background. Columns k..7 go through ONE dma_gather dispatched right
    after the indirect ops, by when the library is resident (~15.1us).
    Risk hedged: if indirect desc-gen stalls behind the Q7 library loader,
    the ops simply run at ~15.1 and total time degrades to ~the old 24.

    g2 is 2D [P, TPP*DIM] (3D dest APs break the indirect expansion);
    dma_gather's dest view is rearranged to [P, m, DIM] on the same bytes.
    Token layout matches the shipped v6 kernel: out[p, j] = token p*8+j.
    """
    assert 1 <= k <= 7
    ng = (TPP - k) * P                    # tokens in the dma_gather half
    orig_barrier = bass.Bass.all_engine_barrier
    bass.Bass.all_engine_barrier = lambda self, *a, **k2: None
    try:
        nc = bacc.Bacc(
            None, target_bir_lowering=False, dynamic_dma_scratch_size=32768
        )
    finally:
        bass.Bass.all_engine_barrier = orig_barrier

    x32 = nc.dram_tensor("x32", [P, k], mybir.dt.int32, kind="ExternalInput")
    xw = nc.dram_tensor("xw", [P, ng // 16], mybir.dt.int16, kind="ExternalInput")
    w = nc.dram_tensor("weight", [VOCAB, DIM], mybir.dt.float32, kind="ExternalInput")
    out = nc.dram_tensor("out", [P, TPP, DIM], mybir.dt.float32, kind="ExternalOutput")

    with contextlib.ExitStack() as ctx:
        idx32_t = ctx.enter_context(nc.sbuf_tensor("idx32", [P, k], mybir.dt.int32))
        idxw_t = ctx.enter_context(
            nc.sbuf_tensor("idxw", [P, ng // 16], mybir.dt.int16)
        )
        g2 = ctx.enter_context(
            nc.sbuf_tensor("g2", [P, TPP * DIM], mybir.dt.float32)
        )
        s_i32 = ctx.enter_context(nc.semaphore("s_i32"))
        s_iw = ctx.enter_context(nc.semaphore("s_iw"))
        s_gi = [ctx.enter_context(nc.semaphore(f"s_gi{j}")) for j in range(k)]
        s_gg = ctx.enter_context(nc.semaphore("s_gg"))
        s_out = ctx.enter_context(nc.semaphore("s_out"))

        nc.gpsimd.load_library(library_config.mlp)
        n_reg = nc.gpsimd.to_reg(ng)

        nc.sync.dma_start(idx32_t[:], x32[:]).then_inc(s_i32, 16)
        nc.sync.dma_start(idxw_t[:], xw[:]).then_inc(s_iw, 16)

        for it in range(loop_m):
            if it > 0:
                nc.sync.wait_ge(s_out, 32 * it)
            nc.gpsimd.wait_ge(s_i32, 16 * (it + 1))
            for j in range(k):
                nc.gpsimd.indirect_dma_start(
                    out=g2[:, j * DIM : (j + 1) * DIM],
                    out_offset=None,
                    in_=w[:],
                    in_offset=bass.IndirectOffsetOnAxis(
                        ap=idx32_t[:, j : j + 1], axis=0
                    ),
                ).then_inc(s_gi[j], 16)
            nc.gpsimd.wait_ge(s_iw, 16 * (it + 1))
            nc.gpsimd.dma_gather(
                g2[:, k * DIM :].rearrange("p (m d) -> p m d", d=DIM),
                w[:],
                idxw_t[:],
                ng,
                n_reg,
                DIM,
                queue_num=0,
            ).then_inc(s_gg, 16)
            for j in range(k):
                nc.sync.wait_ge(s_gi[j], 16 * (it + 1))
            nc.sync.dma_start(
                out[:, 0:k, :], g2[:, 0 : k * DIM]
            ).then_inc(s_out, 16)
            nc.scalar.wait_ge(s_gg, 16 * (it + 1))
            nc.scalar.dma_start(
                out[:, k:, :], g2[:, k * DIM :]
            ).then_inc(s_out, 16)
    nc.compile()
    return nc


def _wrap_idxs(tokens: np.ndarray) -> np.ndarray:
    """dma_gather idx layout: idx[p, s] = tokens[s*16 + p%16], [128, 64] i16."""
    t16 = tokens.reshape(IDX_COLS, 16).T.astype(np.int16)  # [16, 64]
    return np.ascontiguousarray(np.tile(t16, (P // 16, 1)))


_IOTA_WRAPPED = None


def _wrap_iota() -> np.ndarray:
    global _IOTA_WRAPPED
    if _IOTA_WRAPPED is None:
        _IOTA_WRAPPED = _wrap_idxs(np.arange(TPC, dtype=np.int64))
    return _IOTA_WRAPPED


def bench_in_maps(inputs):
    x_flat = inputs["x"].reshape(-1).astype(np.int64)
    w = np.ascontiguousarray(inputs["weight"].astype(np.float32))
    return [
        {
            "x": _wrap_idxs(x_flat[c * TPC : (c + 1) * TPC]),
            "weight": w,
            "wb_idx": _wrap_iota(),
        }
        for c in range(N_CORES)
    ]


def kernel(x: np.ndarray, weight: np.ndarray, **run_kwargs):
    global _NC_CACHE
    if _NC_CACHE is None:
        _NC_CACHE = build_nc_v6(
            shared_idx=True, shared_g=True, n_queues=1,
            wb_groups=((0, 1, 2, 3), (4, 5), (6,), (7,)),
            wb_engines=("sync", "act", "sync", "act"), lean=True,
        )
    nc = _NC_CACHE

    x_flat = np.asarray(x).reshape(-1).astype(np.int64)
    w = np.ascontiguousarray(np.asarray(weight, dtype=np.float32))

    in_maps = [
        {
            # shared_idx layout: x[p, j] = token p*TPP + j (see build_nc_v6)
            "x": np.ascontiguousarray(
                x_flat[c * TPC : (c + 1) * TPC].reshape(P, TPP).astype(np.int32)
            ),
            "weight": w,
        }
        for c in range(N_CORES)
    ]
    res = run_bass_kernel_spmd(nc, in_maps, core_ids=list(range(N_CORES)), **run_kwargs)
    # out [128, 8, 128]: token p*8+j lives at [p, j, :] -> plain reshape
    parts = [res.results[c]["out"].reshape(TPC, DIM) for c in range(N_CORES)]
    full = np.concatenate(parts, axis=0).reshape(B, S, DIM)
    if run_kwargs:
        return full, res
    return full
